# revision 1
# baseline (speedup 1.0000x reference)
"""Trainium2 Bass kernel for LocalDenseSynthesizerAttention.

Data-parallel over batch B=8 -> 8 cores, one batch each. Wire-traffic and
dispatch optimized for the axon tunnel (~90MB/s each way, full duplex):
  - jitted executables built once and cached (no per-call retrace)
  - q shipped t-major fp8 (e4m3) and transposed on-device (PE transpose);
    v shipped t-major bf16 and transposed on-device via XBAR DMA;
    output returned bf16 and widened exactly on host
  - w1/w2 shipped fp8 scaled x16 (rescaled on device via activation scale),
    w3/w_out bf16; shipped as 8-way shards once per call to a tiny
    weights launch that AllGathers them on device; the full per-core
    weights stay device-resident and feed the compute launches
  - compute is split into sequence chunks (the attention window is local,
    halo = 22), one 8-core launch per chunk: chunk i+1's upload overlaps
    chunk i's exec + download
  - donated output buffers created on-device (no zeros upload)

The local window C=45 weighted sum is computed as banded matmuls: the banded
matrix B[s,t'] = attn[t0+t',h,s-t'] is an affine strided view of a zero-padded
attn tensor in DRAM, loaded matmul-ready via XBAR transpose-DMA.

Self-contained: hardcodes shapes from the problem spec.
"""
import sys
sys.path.insert(0, '/opt/trn_rl_repo')
import numpy as np
import ml_dtypes

import concourse.bass as bass
import concourse.mybir as mybir
import concourse.tile as tile
from concourse import bacc
from concourse import masks

T, F = 2048, 512
H, C, DK = 8, 45, 64
HC = H * C          # 360
W = 128             # padded attn width per head (covers s-t' in [-63,127])
S = 64              # t' band-block size
PADV = 22           # (C-1)//2
KF = F // 128       # 4 contraction chunks
B = 8               # total batches / cores
FSH = F // B        # 64 weight-shard rows per core

VH = 64             # v halo rows each side (>= PADV, keeps tiles 128-aligned)
VOFF = VH - PADV    # chunk-vpad[r] = v_logical[r + VOFF]
# (start, length) sequence chunks; lengths must be multiples of 512 and get
# one compiled NEFF each. Two even chunks measured fastest (finer splits pay
# more per-launch overhead than they save in tail).
CHUNKS = [(0, 1024), (1024, 1024)]

BF16 = mybir.dt.bfloat16
FP8 = mybir.dt.float8e4
F32 = mybir.dt.float32
WSCALE = 16.0       # fp8 weight pre-scale for w1/w2

_CACHE = {}


# The build functions are compiled from a synthetic filename so the
# source-location debug info embedded in the BIR (and thus the NEFF
# cache key) does not depend on where this file lives.
_BUILD_SRC = r'''
def _build_w():
    """Tiny weights launch: AllGather 8-way weight shards into full
    per-core weights (device-resident outputs)."""
    nc = bacc.Bacc("TRN2", target_bir_lowering=False, debug=False,
                   num_devices=B, disable_frame_to_traceback=True)
    w1s = nc.dram_tensor("w1s", (FSH, F), FP8, kind="ExternalInput")
    w2s = nc.dram_tensor("w2s", (FSH, HC), FP8, kind="ExternalInput")
    w3s = nc.dram_tensor("w3s", (FSH, F), BF16, kind="ExternalInput")
    wos = nc.dram_tensor("wos", (FSH, F), BF16, kind="ExternalInput")
    w1f = nc.dram_tensor("w1f", (F, F), FP8, kind="ExternalOutput")
    w2f = nc.dram_tensor("w2f", (F, HC), FP8, kind="ExternalOutput")
    w3f = nc.dram_tensor("w3f", (F, F), BF16, kind="ExternalOutput")
    wof = nc.dram_tensor("wof", (F, F), BF16, kind="ExternalOutput")
    groups = [list(range(B))]
    with tile.TileContext(nc) as tc:
        with tc.tile_pool(name="dram", bufs=1, space="DRAM") as dp:
            # collectives cannot read IO tensors: stage shards first
            stages = (dp.tile([FSH, F], FP8, name="st1"),
                      dp.tile([FSH, HC], FP8, name="st2"),
                      dp.tile([FSH, F], BF16, name="st3"),
                      dp.tile([FSH, F], BF16, name="st4"))
            fulls = (dp.tile([F, F], FP8, name="fu1"),
                     dp.tile([F, HC], FP8, name="fu2"),
                     dp.tile([F, F], BF16, name="fu3"),
                     dp.tile([F, F], BF16, name="fu4"))
            for shard, stage, full, out in zip(
                    (w1s, w2s, w3s, wos), stages, fulls,
                    (w1f, w2f, w3f, wof)):
                nc.sync.dma_start(stage[:, :], shard[:, :])
                nc.gpsimd.collective_compute(
                    "AllGather", mybir.AluOpType.bypass, groups,
                    [stage[:, :]], [full[:, :]])
                nc.sync.dma_start(out[:, :], full[:, :])
    nc.compile()
    return nc


def _build_k(TC):
    """Compute launch for one sequence chunk of TC rows."""
    TV = TC + 2 * VH            # logical v rows (main + halo)
    NT128 = TC // 128           # t-tiles in the chunk
    NTV = TV // 128             # t-tiles of the v input (incl halo)
    NB = TC // S                # band blocks
    nc = bacc.Bacc("TRN2", target_bir_lowering=False, debug=False,
                   num_devices=B, disable_frame_to_traceback=True)
    q = nc.dram_tensor("q", (TC, F), FP8, kind="ExternalInput")
    # v is split so the main part exactly matches the output shape/dtype and
    # can be donated/aliased as the output buffer (saves a zeros launch):
    # vh rows [0, VH) = rows just before the chunk, [VH, 2VH) = just after
    v = nc.dram_tensor("v", (TC, F), BF16, kind="ExternalInput")
    vh = nc.dram_tensor("vh", (2 * VH, F), BF16, kind="ExternalInput")
    w1f = nc.dram_tensor("w1f", (F, F), FP8, kind="ExternalInput")
    w2f = nc.dram_tensor("w2f", (F, HC), FP8, kind="ExternalInput")
    w3f = nc.dram_tensor("w3f", (F, F), BF16, kind="ExternalInput")
    wof = nc.dram_tensor("wof", (F, F), BF16, kind="ExternalInput")
    out = nc.dram_tensor("out", (TC, F), BF16, kind="ExternalOutput")

    with tile.TileContext(nc) as tc:
        with tc.tile_pool(name="wpool", bufs=1) as wp, \
             tc.tile_pool(name="inpool", bufs=1) as inp, \
             tc.tile_pool(name="persist", bufs=1) as pers, \
             tc.tile_pool(name="work", bufs=2) as wk, \
             tc.tile_pool(name="band", bufs=4) as bp, \
             tc.tile_pool(name="psmain", bufs=2, space="PSUM") as psm, \
             tc.tile_pool(name="psband", bufs=4, space="PSUM") as psb, \
             tc.tile_pool(name="pstp", bufs=2, space="PSUM") as ptp, \
             tc.tile_pool(name="drampool", bufs=1, space="DRAM") as dp:

            # ---- weights to SBUF, [128, KF, n] layout (partition = contraction)
            w1_t = wp.tile([128, KF, F], FP8, tag="w1")
            nc.sync.dma_start(w1_t[:], w1f[:, :].rearrange("(ko p) n -> p ko n", p=128))
            w2_t = wp.tile([128, KF, HC], FP8, tag="w2")
            nc.sync.dma_start(w2_t[:], w2f[:, :].rearrange("(ko p) n -> p ko n", p=128))
            w3_t = wp.tile([128, KF, F], BF16, tag="w3")
            nc.sync.dma_start(w3_t[:], w3f[:, :].rearrange("(ko p) n -> p ko n", p=128))
            wo_t = wp.tile([128, KF, F], BF16, tag="wo")
            nc.sync.dma_start(wo_t[:], wof[:, :].rearrange("(ko p) n -> p ko n", p=128))

            # ---- v (t-major bf16): XBAR transpose to f-major
            # vT_t cols: [0, VH) front halo | [VH, VH+TC) main | back halo
            vT_t = inp.tile([128, KF, TV], BF16, tag="vT")
            vhT = inp.tile([128, KF, 2 * VH], BF16, tag="vhT")
            for fo in range(KF):
                eng = nc.scalar if fo % 2 else nc.sync
                eng.dma_start_transpose(vT_t[:, fo, VH:VH + TC],
                                        v[:, fo * 128:(fo + 1) * 128])
                eng.dma_start_transpose(vhT[:, fo, :],
                                        vh[:, fo * 128:(fo + 1) * 128])
            nc.vector.tensor_copy(out=vT_t[:, :, 0:VH], in_=vhT[:, :, 0:VH])
            nc.vector.tensor_copy(out=vT_t[:, :, VH + TC:TV],
                                  in_=vhT[:, :, VH:2 * VH])

            # ---- q (t-major fp8): PE-transpose to f-major
            ident = pers.tile([128, 128], FP8, tag="ident")
            masks.make_identity(nc, ident[:])
            qT_t = inp.tile([128, KF, TC], FP8, tag="qT")
            for tt in range(NT128):
                qstage = wk.tile([128, F], FP8, tag="qstage")
                nc.sync.dma_start(qstage[:], q[tt * 128:(tt + 1) * 128, :])
                for fo in range(KF):
                    # fp8 PE transpose requires output element step of 2
                    pst = ptp.tile([128, 256], FP8, tag="qtp")
                    pstv = pst[:].rearrange("p (a b) -> p a b", b=2)[:, :, 0]
                    nc.tensor.transpose(pstv,
                                        qstage[:, fo * 128:(fo + 1) * 128],
                                        ident[:])
                    nc.scalar.copy(qT_t[:, fo, tt * 128:(tt + 1) * 128],
                                   pstv)

            # ---- DRAM scratch
            # vproj rows j = w3-projection of v_in row j; chunk-vpad[r] = row
            # r + VOFF; v_in's zero halo rows project to exact zeros
            vproj = dp.tile([TV, F], BF16)
            # apad: 1 guard row + TC data rows + 1 guard row, row = [8 heads x 128]
            apad = dp.tile([TC + 2, H * W], BF16)

            # zero tile for apad guards
            z_t = pers.tile([128, H * W], BF16, tag="zt")
            nc.any.memzero(z_t[:])
            nc.sync.dma_start(apad[0:1, :], z_t[0:1, :])
            nc.sync.dma_start(apad[TC + 1:TC + 2, :], z_t[0:1, :])

            # ---- persistent SBUF activations
            qrT = pers.tile([128, KF, TC], FP8, tag="qrT")   # relu(q @ w1), f-major
            xT = pers.tile([128, KF, TC], BF16, tag="xT")    # band output, f-major

            # ================= Phase A: q-proj + relu (f-major out) ===========
            # PSUM = q @ (16 w1); Relu(psum/16) -> fp8
            for fo in range(KF):
                for tt in range(TC // 512):
                    ps = psm.tile([128, 512], F32, tag="mm")
                    for k in range(KF):
                        nc.tensor.matmul(
                            ps[:], w1_t[:, k, fo * 128:(fo + 1) * 128],
                            qT_t[:, k, tt * 512:(tt + 1) * 512],
                            start=(k == 0), stop=(k == KF - 1))
                    nc.scalar.activation(qrT[:, fo, tt * 512:(tt + 1) * 512], ps[:],
                                         mybir.ActivationFunctionType.Relu,
                                         scale=1.0 / WSCALE)

            # ================= Phase C: v-proj (t-major out) -> vproj =========
            for tb in range(NTV):
                ps = psm.tile([128, 512], F32, tag="mm")
                for k in range(KF):
                    nc.tensor.matmul(
                        ps[:], vT_t[:, k, tb * 128:(tb + 1) * 128],
                        w3_t[:, k, :],
                        start=(k == 0), stop=(k == KF - 1))
                v_sb = wk.tile([128, F], BF16, tag="vsb")
                nc.scalar.copy(v_sb[:], ps[:])
                nc.sync.dma_start(vproj[tb * 128:(tb + 1) * 128, :], v_sb[:])

            # ====== Phase B: s-proj (t-major) + softmax -> apad (padded) ======
            # PSUM = qr @ (16 w2); Exp(psum/16)
            for tb in range(NT128):
                ps = psm.tile([128, 512], F32, tag="mm")
                for k in range(KF):
                    nc.tensor.matmul(
                        ps[:, 0:HC], qrT[:, k, tb * 128:(tb + 1) * 128],
                        w2_t[:, k, :],
                        start=(k == 0), stop=(k == KF - 1))
                e_t = wk.tile([128, HC], F32, tag="et")
                nc.scalar.activation(e_t[:], ps[:, 0:HC],
                                     mybir.ActivationFunctionType.Exp,
                                     scale=1.0 / WSCALE)
                zs = wk.tile([128, H], F32, tag="zs")
                nc.vector.reduce_sum(zs[:], e_t[:].rearrange("p (h c) -> p h c", c=C),
                                     axis=mybir.AxisListType.X)
                rz = wk.tile([128, H], F32, tag="rz")
                nc.vector.reciprocal(rz[:], zs[:])
                ap_t = wk.tile([128, H * W], BF16, tag="apad")
                if tb < 2:
                    # zero the pad region once per pool slot (bufs=2); the pad
                    # columns are never overwritten afterwards
                    nc.any.memzero(ap_t[:])
                nc.vector.tensor_mul(
                    out=ap_t[:].rearrange("p (h w) -> p h w", w=W)[:, :, 0:C],
                    in0=e_t[:].rearrange("p (h c) -> p h c", c=C),
                    in1=rz[:, :, None].to_broadcast((128, H, C)))
                nc.sync.dma_start(apad[1 + tb * 128:1 + (tb + 1) * 128, :], ap_t[:])

            # ================= Phase D: banded attention matmuls ==============
            # x[t', h*64+d] = sum_s chunkvpad[t0+s, h*64+d] * B_h[s, t']
            # B_h loaded via transpose-DMA of sheared apad view.
            apad_h = apad.tensor  # underlying DRAM handle
            apad_off = apad.offset if isinstance(apad.offset, int) else 0
            for g in range(NB // 4):    # groups of 4 band blocks = 256 t'
                pss = [psb.tile([128, 512], F32, tag="px", name=f"px{g}_{pi}")
                       for pi in range(4)]
                for j in range(4):
                    bi = g * 4 + j
                    t0 = S * bi
                    vsp = wk.tile([128, F], BF16, tag="vsp")
                    nc.sync.dma_start(vsp[:], vproj[VOFF + t0:VOFF + t0 + 128, :])
                    for p in range(4):      # head pairs
                        for i in range(2):
                            h = 2 * p + i
                            b_t = bp.tile([W, S], BF16, tag="bt")
                            src = bass.AP(
                                tensor=apad_h,
                                offset=apad_off + (1 + t0) * (H * W) + h * W,
                                ap=[[H * W - 1, S], [1, W]])
                            eng = nc.scalar if h % 2 else nc.sync
                            eng.dma_start_transpose(b_t[:], src)
                            # lhsT = v head-pair [128, 128]; valid out rows are
                            # [i*64:(i+1)*64]; the other half is garbage and
                            # ignored at copyback.
                            nc.tensor.matmul(
                                pss[p][:, j * 128 + i * 64: j * 128 + (i + 1) * 64],
                                vsp[:, p * 128:(p + 1) * 128], b_t[:],
                                start=True, stop=True)
                # copy valid quadrants -> xT (f-major): fold p rows 0:63 = head
                # 2p (cols i=0), rows 64:127 = head 2p+1 (cols i=1)
                for p in range(4):
                    ps3 = pss[p][:].rearrange("d (j i k) -> d j i k", j=4, i=2)
                    dst = xT[:, p, g * 256:(g + 1) * 256] \
                        .rearrange("d (j k) -> d j k", j=4)
                    nc.vector.tensor_copy(out=dst[0:64], in_=ps3[0:64, :, 0, :])
                    nc.vector.tensor_copy(out=dst[64:128], in_=ps3[64:128, :, 1, :])

            # ================= Phase E: out-proj (t-major out) ================
            for tb in range(NT128):
                ps = psm.tile([128, 512], F32, tag="mm")
                for k in range(KF):
                    nc.tensor.matmul(
                        ps[:], xT[:, k, tb * 128:(tb + 1) * 128],
                        wo_t[:, k, :],
                        start=(k == 0), stop=(k == KF - 1))
                o_sb = wk.tile([128, F], BF16, tag="osb")
                nc.scalar.copy(o_sb[:], ps[:])
                nc.sync.dma_start(out[tb * 128:(tb + 1) * 128, :], o_sb[:])

    nc.compile()
    return nc
'''

exec(compile(_BUILD_SRC, "bass_build_k", "exec"), globals())


def _make_exec(nc, devices, donate_input=None):
    """Cached jitted executable + on-device zeros maker for one bass module.

    With donate_input=<name>, that input is donated and XLA aliases its
    buffer as the (shape/dtype-matching) output — no zero buffers needed."""
    import jax
    import jax.numpy as jnp
    from jax.sharding import Mesh, PartitionSpec, NamedSharding
    from jax.experimental.shard_map import shard_map
    from concourse.bass2jax import _bass_exec_p, partition_id_tensor

    partition_name = (nc.partition_id_tensor.name
                      if nc.partition_id_tensor else None)
    in_names, out_names, out_avals = [], [], []
    for alloc in nc.m.functions[0].allocations:
        if not isinstance(alloc, mybir.MemoryLocationSet):
            continue
        if alloc.kind not in ("ExternalInput", "ExternalOutput"):
            continue
        name = alloc.memorylocations[0].name
        if alloc.kind == "ExternalInput":
            if name != partition_name:
                in_names.append(name)
        else:
            out_avals.append(jax.core.ShapedArray(
                tuple(alloc.tensor_shape), mybir.dt.np(alloc.dtype)))
            out_names.append(name)
    n_params, n_outs = len(in_names), len(out_avals)
    in_names_all = list(in_names) + list(out_names)
    if partition_name is not None:
        in_names_all.append(partition_name)

    def _body(*args):
        operands = list(args)
        if partition_name is not None:
            operands.append(partition_id_tensor())
        return tuple(_bass_exec_p.bind(
            *operands,
            out_avals=tuple(out_avals),
            in_names=tuple(in_names_all),
            out_names=tuple(out_names),
            lowering_input_output_aliases=(),
            sim_require_finite=True,
            sim_require_nnan=True,
            nc=nc))

    n = len(devices)
    mesh = Mesh(np.asarray(devices), ("core",))
    shard = NamedSharding(mesh, PartitionSpec("core"))
    if donate_input is None:
        n_args = n_params + n_outs
        donate = tuple(range(n_params, n_args))
        mkzeros = jax.jit(
            lambda: tuple(jnp.zeros((n * a.shape[0], *a.shape[1:]), a.dtype)
                          for a in out_avals),
            out_shardings=(shard,) * n_outs)
        body = _body
    else:
        # outputs alias the donated input's buffer; no zero operands
        n_args = n_params
        donate = (in_names.index(donate_input),)
        mkzeros = None
        in_names_all[:] = list(in_names)
        if partition_name is not None:
            in_names_all.append(partition_name)
        body = _body
    in_specs = (PartitionSpec("core"),) * n_args
    out_specs = (PartitionSpec("core"),) * n_outs
    sharded = jax.jit(
        shard_map(body, mesh=mesh, in_specs=in_specs, out_specs=out_specs,
                  check_rep=False),
        donate_argnums=donate, keep_unused=True)
    return {"sharded": sharded, "mkzeros": mkzeros, "in_names": in_names,
            "out_names": out_names, "shard": shard, "n": n}


def _get_state():
    if "state" in _CACHE:
        return _CACHE["state"]
    import jax
    from concourse.bass2jax import install_neuronx_cc_hook
    install_neuronx_cc_hook()
    devices = jax.devices()[:B]
    wexec = _make_exec(_build_w(), devices)
    kexecs = {}
    for _, tc in CHUNKS:
        if tc not in kexecs:
            kexecs[tc] = _make_exec(_build_k(tc), devices, donate_input="v")
    state = {"w": wexec, "k": kexecs}
    _CACHE["state"] = state
    return state


def _to_bf16_bits(x32):
    """fp32 -> bf16 via round-half-up on the upper 16 bits (RNE-grade error,
    much faster than ml_dtypes astype). Returns uint16 bit pattern."""
    if x32.strides[-1] != 4:
        x32 = np.ascontiguousarray(x32)
    tmp = x32.view(np.uint32) + np.uint32(0x8000)
    np.right_shift(tmp, 16, out=tmp)
    return tmp.astype(np.uint16)


def kernel(query, key, value, w1, w2, w3, w_out, _trace=False):
    out, ok = _kernel_once(query, key, value, w1, w2, w3, w_out)
    # The remote runtime very occasionally returns a stale/corrupt buffer
    # (output absmax ~0.16 for this problem; leftover v data is ~5).
    # Retry once on implausible output.
    if not ok:
        out, ok = _kernel_once(query, key, value, w1, w2, w3, w_out)
    return out


def _kernel_once(query, key, value, w1, w2, w3, w_out):
    # Host has a single CPU core: interleave each (cheap) conversion with the
    # async uploads so the network stays busy from ~40ms in.
    import jax
    st = _get_state()
    e4 = ml_dtypes.float8_e4m3
    bf = ml_dtypes.bfloat16
    wx = st["w"]

    query = np.asarray(query)
    value = np.asarray(value)

    # device-side zero buffers for the weights launch: dispatch first (no
    # uplink bytes; its RPC latency runs under the v0 conversion)
    wzeros = wx["mkzeros"]()

    # ---- per-chunk compute launches, pipelined; convert -> put per tensor
    pending = []
    wfull = None
    for i, (c0, tc) in enumerate(CHUNKS):
        kx = st["k"][tc]
        # v main part first (biggest upload; donated -> output buffer)
        vb = _to_bf16_bits(value[:, c0:c0 + tc]).view(bf).reshape(B * tc, F)
        v_dev = jax.device_put(vb, kx["shard"])
        if i == 0:
            # weights launch: tiny upload queued behind v0; its exec (RPC)
            # overlaps the remaining uploads and finishes before K0 starts
            warrs = {"w1s": (np.asarray(w1) * WSCALE).astype(e4),
                     "w2s": (np.asarray(w2) * WSCALE).astype(e4),
                     "w3s": _to_bf16_bits(np.asarray(w3)).view(bf),
                     "wos": _to_bf16_bits(np.asarray(w_out)).view(bf)}
            wouts = wx["sharded"](*[warrs[n] for n in wx["in_names"]],
                                  *wzeros)
            wfull = dict(zip(wx["out_names"], wouts))
        # v halo rows (zero-padded at sequence edges)
        hbuf = np.zeros((B, 2 * VH, F), np.uint16)
        if c0 > 0:
            hbuf[:, 0:VH] = _to_bf16_bits(value[:, c0 - VH:c0])
        if c0 + tc < T:
            hbuf[:, VH:2 * VH] = _to_bf16_bits(value[:, c0 + tc:c0 + tc + VH])
        vh_dev = jax.device_put(hbuf.view(bf).reshape(B * 2 * VH, F),
                                kx["shard"])
        # q chunk, t-major fp8 (transposed on device)
        q8 = query[:, c0:c0 + tc, :].astype(e4).reshape(B * tc, F)
        q_dev = jax.device_put(q8, kx["shard"])
        arrays = {"q": q_dev, "v": v_dev, "vh": vh_dev, "w1f": wfull["w1f"],
                  "w2f": wfull["w2f"], "w3f": wfull["w3f"],
                  "wof": wfull["wof"]}
        ins = [arrays[n] for n in kx["in_names"]]
        outs = kx["sharded"](*ins)
        outs[0].copy_to_host_async()   # start D2H as soon as exec finishes
        pending.append(outs)

    # ---- collect: widen bf16 -> fp32 exactly (zero-extension)
    buf = np.zeros((B, T, F, 2), np.uint16)
    for (c0, tc), outs in zip(CHUNKS, pending):
        o16 = np.asarray(outs[0]).view(np.uint16).reshape(B, tc, F)
        buf[:, c0:c0 + tc, :, 1] = o16
    # plausibility check on the bf16 bits (contiguous, much cheaper than
    # abs().max() on the strided fp32 view): for bf16, |x| > 2.0 iff
    # (bits & 0x7FFF) > 0x4000; NaN/Inf patterns (>= 0x7F80) also exceed it
    m = np.bitwise_and(buf[:, :, :, 1], np.uint16(0x7FFF)).max()
    return buf.view(np.float32)[..., 0], m <= 0x4000



# revision 5
# speedup vs baseline: 1.6671x; 1.6671x over previous
"""Trainium2 Bass kernel for LocalDenseSynthesizerAttention.

Data-parallel over batch B=8 -> 8 cores, one batch each. The axon tunnel
(~45MB/s, effectively half-duplex) dominates, so the design minimizes wire
bytes:
  - q and v shipped int8 with per-row (per-t) scales; scales are bf16-rounded
    on host so host and device use bit-identical values, then shipped f32
  - v ships with a 64-row halo folded into the same array (window is local,
    pad = 22), so no separate halo tensor or device-side halo assembly
  - output quantized to int8 with per-row scales ON DEVICE (row absmax ->
    126/absmax), downloaded as int8 + f32 scales, dequantized on host.
    Device rounding uses the +1.5*2^23 magic-number trick so the result is
    exactly np.rint regardless of the convert instruction's rounding mode.
  - projection weights shipped f32 ONCE (content-compared per call, reuses
    device-resident copies), AllGathered from 8-way shards on device, stored
    pre-transposed [128, KF, N] fp16 for the compute launches
  - device compute in fp16 (PE supports fp16 matmul) instead of bf16: the
    extra mantissa bits keep total rel-err at baseline level (~1.0e-2)
    despite int8 I/O
  - q+v packed into one int8 blob per chunk (fewer device_puts; each put has
    a large fixed cost), puts dispatched from a small thread pool
  - compute split into sequence chunks, one 8-core launch per chunk, so host
    quantization overlaps wire transfer

The local window C=45 weighted sum is computed as banded matmuls: the banded
matrix B[s,t'] = attn[t0+t',h,s-t'] is an affine strided view of a zero-padded
attn tensor in DRAM, loaded matmul-ready via XBAR transpose-DMA.

Self-contained: hardcodes shapes from the problem spec.
"""
import sys
sys.path.insert(0, '/opt/trn_rl_repo')
import numpy as np

import concourse.bass as bass
import concourse.mybir as mybir
import concourse.tile as tile
from concourse import bacc
from concourse import masks

T, F = 2048, 512
H, C, DK = 8, 45, 64
HC = H * C          # 360
W = 128             # padded attn width per head (covers s-t' in [-63,127])
S = 64              # t' band-block size
PADV = 22           # (C-1)//2
KF = F // 128       # 4 contraction chunks
B = 8               # total batches / cores
FSH = F // B        # 64 weight-shard rows per core

VH = 64             # v halo rows each side (>= PADV, keeps tiles 128-aligned)
VOFF = VH - PADV    # chunk-vpad[r] = v_logical[r + VOFF]
CHUNKS = [(0, 1024), (1024, 1024)]

F16 = mybir.dt.float16
F32 = mybir.dt.float32
I8 = mybir.dt.int8
QD = 126.0          # int8 quant denominator (126 leaves headroom for the
                    # bf16 round-down of the scale: 126*1.002 < 126.5)
MAGIC = 12582912.0  # 1.5 * 2^23: fp32 add rounds the value to nearest int

_CACHE = {}


# The build functions are compiled from a synthetic filename so the
# source-location debug info embedded in the BIR (and thus the NEFF
# cache key) does not depend on where this file lives.
_BUILD_SRC = r'''
def _build_w():
    """Weights launch (first call only): AllGather 8-way f32 weight shards,
    convert to fp16 in the matmul-ready [128, KF, N] layout, store to
    device-resident DRAM outputs."""
    nc = bacc.Bacc("TRN2", target_bir_lowering=False, debug=False,
                   num_devices=B, disable_frame_to_traceback=True)
    w1s = nc.dram_tensor("w1s", (FSH, F), F32, kind="ExternalInput")
    w2s = nc.dram_tensor("w2s", (FSH, HC), F32, kind="ExternalInput")
    w3s = nc.dram_tensor("w3s", (FSH, F), F32, kind="ExternalInput")
    wos = nc.dram_tensor("wos", (FSH, F), F32, kind="ExternalInput")
    w1f = nc.dram_tensor("w1f", (128, KF * F), F16, kind="ExternalOutput")
    w2f = nc.dram_tensor("w2f", (128, KF * HC), F16, kind="ExternalOutput")
    w3f = nc.dram_tensor("w3f", (128, KF * F), F16, kind="ExternalOutput")
    wof = nc.dram_tensor("wof", (128, KF * F), F16, kind="ExternalOutput")
    groups = [list(range(B))]
    with tile.TileContext(nc) as tc:
        with tc.tile_pool(name="dram", bufs=1, space="DRAM") as dp, \
             tc.tile_pool(name="sb", bufs=2) as sp:
            for idx, (shard, out, n) in enumerate(
                    ((w1s, w1f, F), (w2s, w2f, HC),
                     (w3s, w3f, F), (wos, wof, F))):
                stage = dp.tile([FSH, n], F32, name=f"st{idx}")
                full = dp.tile([F, n], F32, name=f"fu{idx}")
                # collectives cannot read IO tensors: stage shards first
                nc.sync.dma_start(stage[:, :], shard[:, :])
                nc.gpsimd.collective_compute(
                    "AllGather", mybir.AluOpType.bypass, groups,
                    [stage[:, :]], [full[:, :]])
                sb32 = sp.tile([128, KF, n], F32, tag=f"sb32_{n}")
                nc.sync.dma_start(
                    sb32[:], full[:, :].rearrange("(ko p) n -> p ko n", p=128))
                sb16 = sp.tile([128, KF, n], F16, tag=f"sb16_{n}")
                nc.scalar.copy(sb16[:], sb32[:])
                nc.sync.dma_start(
                    out[:, :].rearrange("p (ko n) -> p ko n", ko=KF), sb16[:])
    nc.compile()
    return nc


def _build_k(TC):
    """Compute launch for one sequence chunk of TC rows."""
    TCV = TC + 2 * VH           # v rows incl halo
    NT = TC // 128              # t-tiles in the chunk
    NTV = TCV // 128            # v tiles incl halo
    NB = TC // S                # band blocks
    nc = bacc.Bacc("TRN2", target_bir_lowering=False, debug=False,
                   num_devices=B, disable_frame_to_traceback=True)
    # blob = q int8 rows (TC,F) then v int8 rows (TCV,F), flat
    blob = nc.dram_tensor("blob", (TC * F + TCV * F,), I8,
                          kind="ExternalInput")
    # scl = q row scales (TC) then v row scales (TCV)
    scl = nc.dram_tensor("scl", (TC + TCV, 1), F32, kind="ExternalInput")
    w1f = nc.dram_tensor("w1f", (128, KF * F), F16, kind="ExternalInput")
    w2f = nc.dram_tensor("w2f", (128, KF * HC), F16, kind="ExternalInput")
    w3f = nc.dram_tensor("w3f", (128, KF * F), F16, kind="ExternalInput")
    wof = nc.dram_tensor("wof", (128, KF * F), F16, kind="ExternalInput")
    oq = nc.dram_tensor("oq", (TC, F), I8, kind="ExternalOutput")
    os_ = nc.dram_tensor("os", (TC, 1), F32, kind="ExternalOutput")

    with tile.TileContext(nc) as tc:
        with tc.tile_pool(name="wpool", bufs=1) as wp, \
             tc.tile_pool(name="inpool", bufs=1) as inp, \
             tc.tile_pool(name="persist", bufs=1) as pers, \
             tc.tile_pool(name="work", bufs=2) as wk, \
             tc.tile_pool(name="band", bufs=4) as bp, \
             tc.tile_pool(name="psmain", bufs=2, space="PSUM") as psm, \
             tc.tile_pool(name="psband", bufs=4, space="PSUM") as psb, \
             tc.tile_pool(name="pstp", bufs=2, space="PSUM") as ptp, \
             tc.tile_pool(name="drampool", bufs=1, space="DRAM") as dp:

            # ---- weights to SBUF, [128, KF, n] fp16 (partition = contraction)
            w1_t = wp.tile([128, KF, F], F16, tag="w1")
            nc.sync.dma_start(
                w1_t[:], w1f[:, :].rearrange("p (ko n) -> p ko n", ko=KF))
            w2_t = wp.tile([128, KF, HC], F16, tag="w2")
            nc.sync.dma_start(
                w2_t[:], w2f[:, :].rearrange("p (ko n) -> p ko n", ko=KF))
            w3_t = wp.tile([128, KF, F], F16, tag="w3")
            nc.sync.dma_start(
                w3_t[:], w3f[:, :].rearrange("p (ko n) -> p ko n", ko=KF))
            wo_t = wp.tile([128, KF, F], F16, tag="wo")
            nc.sync.dma_start(
                wo_t[:], wof[:, :].rearrange("p (ko n) -> p ko n", ko=KF))

            ident = pers.tile([128, 128], F16, tag="ident")
            masks.make_identity(nc, ident[:])

            # ---- dequantize q and v (t-major int8 -> fp16), PE-transpose to
            # f-major [128 f, KF, t]
            qT = inp.tile([128, KF, TC], F16, tag="qT")
            vT = inp.tile([128, KF, TCV], F16, tag="vT")
            for dst, nt, boff, soff in ((qT, NT, 0, 0),
                                        (vT, NTV, TC * F, TC)):
                for tt in range(nt):
                    i8 = wk.tile([128, F], I8, tag="i8")
                    src = blob[boff + tt * 128 * F:
                               boff + (tt + 1) * 128 * F]
                    nc.sync.dma_start(
                        i8[:], src.rearrange("(p n) -> p n", n=F))
                    sq = wk.tile([128, 1], F32, tag="sq")
                    nc.sync.dma_start(
                        sq[:], scl[soff + tt * 128:soff + (tt + 1) * 128, :])
                    dq = wk.tile([128, F], F16, tag="dq")
                    nc.scalar.activation(dq[:], i8[:],
                                         mybir.ActivationFunctionType.Copy,
                                         scale=sq[:, :])
                    for fo in range(KF):
                        pst = ptp.tile([128, 128], F16, tag="tp")
                        nc.tensor.transpose(
                            pst[:], dq[:, fo * 128:(fo + 1) * 128], ident[:])
                        nc.scalar.copy(dst[:, fo, tt * 128:(tt + 1) * 128],
                                       pst[:])

            # ---- DRAM scratch
            # vproj rows j = w3-projection of v_in row j; v rows outside the
            # sequence are int8 zeros (scale 1) and project to exact zeros
            vproj = dp.tile([TCV, F], F16)
            # apad: 1 guard row + TC data rows + 1 guard row, row = [8 x 128]
            apad = dp.tile([TC + 2, H * W], F16)

            # zero tile for apad guards
            z_t = pers.tile([128, H * W], F16, tag="zt")
            nc.any.memzero(z_t[:])
            nc.sync.dma_start(apad[0:1, :], z_t[0:1, :])
            nc.sync.dma_start(apad[TC + 1:TC + 2, :], z_t[0:1, :])

            # ---- persistent SBUF activations
            qrT = pers.tile([128, KF, TC], F16, tag="qrT")  # relu(q@w1) f-major
            xT = pers.tile([128, KF, TC], F16, tag="xT")    # band out, f-major

            # ================= Phase A: q-proj + relu (f-major out) ===========
            for fo in range(KF):
                for tt in range(TC // 512):
                    ps = psm.tile([128, 512], F32, tag="mm")
                    for k in range(KF):
                        nc.tensor.matmul(
                            ps[:], w1_t[:, k, fo * 128:(fo + 1) * 128],
                            qT[:, k, tt * 512:(tt + 1) * 512],
                            start=(k == 0), stop=(k == KF - 1))
                    nc.scalar.activation(qrT[:, fo, tt * 512:(tt + 1) * 512],
                                         ps[:],
                                         mybir.ActivationFunctionType.Relu)

            # ================= Phase C: v-proj (t-major out) -> vproj =========
            for tb in range(NTV):
                ps = psm.tile([128, 512], F32, tag="mm")
                for k in range(KF):
                    nc.tensor.matmul(
                        ps[:], vT[:, k, tb * 128:(tb + 1) * 128],
                        w3_t[:, k, :],
                        start=(k == 0), stop=(k == KF - 1))
                v_sb = wk.tile([128, F], F16, tag="vsb")
                nc.scalar.copy(v_sb[:], ps[:])
                nc.sync.dma_start(vproj[tb * 128:(tb + 1) * 128, :], v_sb[:])

            # ====== Phase B: s-proj (t-major) + softmax -> apad (padded) ======
            for tb in range(NT):
                ps = psm.tile([128, 512], F32, tag="mm")
                for k in range(KF):
                    nc.tensor.matmul(
                        ps[:, 0:HC], qrT[:, k, tb * 128:(tb + 1) * 128],
                        w2_t[:, k, :],
                        start=(k == 0), stop=(k == KF - 1))
                e_t = wk.tile([128, HC], F32, tag="et")
                nc.scalar.activation(e_t[:], ps[:, 0:HC],
                                     mybir.ActivationFunctionType.Exp)
                zs = wk.tile([128, H], F32, tag="zs")
                nc.vector.reduce_sum(zs[:],
                                     e_t[:].rearrange("p (h c) -> p h c", c=C),
                                     axis=mybir.AxisListType.X)
                rz = wk.tile([128, H], F32, tag="rz")
                nc.vector.reciprocal(rz[:], zs[:])
                ap_t = wk.tile([128, H * W], F16, tag="apad")
                if tb < 2:
                    # zero the pad region once per pool slot (bufs=2); the pad
                    # columns are never overwritten afterwards
                    nc.any.memzero(ap_t[:])
                nc.vector.tensor_mul(
                    out=ap_t[:].rearrange("p (h w) -> p h w", w=W)[:, :, 0:C],
                    in0=e_t[:].rearrange("p (h c) -> p h c", c=C),
                    in1=rz[:, :, None].to_broadcast((128, H, C)))
                nc.sync.dma_start(apad[1 + tb * 128:1 + (tb + 1) * 128, :],
                                  ap_t[:])

            # ================= Phase D: banded attention matmuls ==============
            # x[t', h*64+d] = sum_s vproj[VOFF+t0+s, h*64+d] * B_h[s, t']
            # B_h loaded via transpose-DMA of sheared apad view.
            apad_h = apad.tensor  # underlying DRAM handle
            apad_off = apad.offset if isinstance(apad.offset, int) else 0
            for g in range(NB // 4):    # groups of 4 band blocks = 256 t'
                pss = [psb.tile([128, 512], F32, tag="px", name=f"px{g}_{pi}")
                       for pi in range(4)]
                for j in range(4):
                    bi = g * 4 + j
                    t0 = S * bi
                    vsp = wk.tile([128, F], F16, tag="vsp")
                    nc.sync.dma_start(vsp[:],
                                      vproj[VOFF + t0:VOFF + t0 + 128, :])
                    for p in range(4):      # head pairs
                        for i in range(2):
                            h = 2 * p + i
                            b_t = bp.tile([W, S], F16, tag="bt")
                            src = bass.AP(
                                tensor=apad_h,
                                offset=apad_off + (1 + t0) * (H * W) + h * W,
                                ap=[[H * W - 1, S], [1, W]])
                            eng = nc.scalar if h % 2 else nc.sync
                            eng.dma_start_transpose(b_t[:], src)
                            # lhsT = v head-pair [128, 128]; valid out rows are
                            # [i*64:(i+1)*64]; the other half is garbage and
                            # ignored at copyback.
                            nc.tensor.matmul(
                                pss[p][:, j * 128 + i * 64:
                                       j * 128 + (i + 1) * 64],
                                vsp[:, p * 128:(p + 1) * 128], b_t[:],
                                start=True, stop=True)
                # copy valid quadrants -> xT (f-major): fold p rows 0:63 = head
                # 2p (cols i=0), rows 64:127 = head 2p+1 (cols i=1)
                for p in range(4):
                    ps3 = pss[p][:].rearrange("d (j i k) -> d j i k", j=4, i=2)
                    dst = xT[:, p, g * 256:(g + 1) * 256] \
                        .rearrange("d (j k) -> d j k", j=4)
                    nc.vector.tensor_copy(out=dst[0:64], in_=ps3[0:64, :, 0, :])
                    nc.vector.tensor_copy(out=dst[64:128],
                                          in_=ps3[64:128, :, 1, :])

            # ========= Phase E: out-proj + per-row int8 quantization ==========
            for tb in range(NT):
                ps = psm.tile([128, 512], F32, tag="mm")
                for k in range(KF):
                    nc.tensor.matmul(
                        ps[:], xT[:, k, tb * 128:(tb + 1) * 128],
                        wo_t[:, k, :],
                        start=(k == 0), stop=(k == KF - 1))
                am = wk.tile([128, 1], F32, tag="am")
                nc.vector.reduce_max(am[:], ps[:], axis=mybir.AxisListType.X,
                                     apply_absolute_value=True)
                rz = wk.tile([128, 1], F32, tag="orz")
                nc.vector.reciprocal(rz[:], am[:])
                rs = wk.tile([128, 1], F32, tag="ors")
                nc.vector.tensor_scalar_mul(rs[:], rz[:], QD)
                y = wk.tile([128, F], F32, tag="oy")
                nc.scalar.activation(y[:], ps[:],
                                     mybir.ActivationFunctionType.Copy,
                                     scale=rs[:, :])
                # round to nearest int (RNE) via magic add/sub, then convert:
                # the value is exactly integral so the convert's rounding
                # mode is irrelevant
                yr = wk.tile([128, F], F32, tag="oyr")
                nc.vector.tensor_scalar(yr[:], y[:], MAGIC, -MAGIC,
                                        mybir.AluOpType.add,
                                        mybir.AluOpType.add)
                oqt = wk.tile([128, F], I8, tag="oqt")
                nc.vector.tensor_copy(out=oqt[:], in_=yr[:])
                ost = wk.tile([128, 1], F32, tag="ost")
                nc.vector.tensor_scalar_mul(ost[:], am[:], 1.0 / QD)
                nc.sync.dma_start(oq[tb * 128:(tb + 1) * 128, :], oqt[:])
                nc.scalar.dma_start(os_[tb * 128:(tb + 1) * 128, :], ost[:])

    nc.compile()
    return nc
'''

exec(compile(_BUILD_SRC, "bass_build_k", "exec"), globals())


def _make_exec(nc, devices):
    """Cached jitted executable for one bass module; outputs come from
    donated on-device zero buffers (mkzeros)."""
    import jax
    import jax.numpy as jnp
    from jax.sharding import Mesh, PartitionSpec, NamedSharding
    from jax.experimental.shard_map import shard_map
    from concourse.bass2jax import _bass_exec_p, partition_id_tensor

    partition_name = (nc.partition_id_tensor.name
                      if nc.partition_id_tensor else None)
    in_names, out_names, out_avals = [], [], []
    for alloc in nc.m.functions[0].allocations:
        if not isinstance(alloc, mybir.MemoryLocationSet):
            continue
        if alloc.kind not in ("ExternalInput", "ExternalOutput"):
            continue
        name = alloc.memorylocations[0].name
        if alloc.kind == "ExternalInput":
            if name != partition_name:
                in_names.append(name)
        else:
            out_avals.append(jax.core.ShapedArray(
                tuple(alloc.tensor_shape), mybir.dt.np(alloc.dtype)))
            out_names.append(name)
    n_params, n_outs = len(in_names), len(out_avals)
    in_names_all = list(in_names) + list(out_names)
    if partition_name is not None:
        in_names_all.append(partition_name)

    def _body(*args):
        operands = list(args)
        if partition_name is not None:
            operands.append(partition_id_tensor())
        return tuple(_bass_exec_p.bind(
            *operands,
            out_avals=tuple(out_avals),
            in_names=tuple(in_names_all),
            out_names=tuple(out_names),
            lowering_input_output_aliases=(),
            sim_require_finite=True,
            sim_require_nnan=True,
            nc=nc))

    n = len(devices)
    mesh = Mesh(np.asarray(devices), ("core",))
    shard = NamedSharding(mesh, PartitionSpec("core"))
    n_args = n_params + n_outs
    donate = tuple(range(n_params, n_args))
    mkzeros = jax.jit(
        lambda: tuple(jnp.zeros((n * a.shape[0], *a.shape[1:]), a.dtype)
                      for a in out_avals),
        out_shardings=(shard,) * n_outs)
    in_specs = (PartitionSpec("core"),) * n_args
    out_specs = (PartitionSpec("core"),) * n_outs
    sharded = jax.jit(
        shard_map(_body, mesh=mesh, in_specs=in_specs, out_specs=out_specs,
                  check_rep=False),
        donate_argnums=donate, keep_unused=True)
    return {"sharded": sharded, "mkzeros": mkzeros, "in_names": in_names,
            "out_names": out_names, "shard": shard, "n": n}


def _get_state():
    if "state" in _CACHE:
        return _CACHE["state"]
    import jax
    from concurrent.futures import ThreadPoolExecutor
    from concourse.bass2jax import install_neuronx_cc_hook
    install_neuronx_cc_hook()
    devices = jax.devices()[:B]
    wexec = _make_exec(_build_w(), devices)
    kexecs = {}
    for _, tc in CHUNKS:
        if tc not in kexecs:
            kexecs[tc] = _make_exec(_build_k(tc), devices)
    state = {"w": wexec, "k": kexecs,
             "pool": ThreadPoolExecutor(max_workers=3)}
    _CACHE["state"] = state
    return state


def _to_bf16_f32(x32):
    """fp32 -> bf16 via round-half-up on the upper 16 bits, returned as
    exact f32 values (so host and device share bit-identical scales)."""
    x32 = np.ascontiguousarray(x32, np.float32)
    tmp = x32.view(np.uint32) + np.uint32(0x8000)
    np.bitwise_and(tmp, np.uint32(0xFFFF0000), out=tmp)
    return tmp.view(np.float32)


def _quant_rows(x):
    """Per-row symmetric int8 quantization of (..., F) fp32.
    Returns (int8 codes, f32 scales[...]) with scales bf16-exact."""
    a = np.abs(x).max(axis=-1)
    np.maximum(a, 1e-30, out=a)
    s = _to_bf16_f32(a / QD)
    y = x * (1.0 / s)[..., None]
    np.rint(y, out=y)
    return y.astype(np.int8), s


def _weights_device(st, w1, w2, w3, w_out):
    """Device-resident fp16 weights, re-uploaded only when contents change."""
    import jax
    ws = [np.ascontiguousarray(np.asarray(w), np.float32)
          for w in (w1, w2, w3, w_out)]
    cached = _CACHE.get("wfull")
    if cached is not None and all(
            np.array_equal(a, b) for a, b in zip(ws, _CACHE["whost"])):
        return cached
    wx = st["w"]
    wzeros = wx["mkzeros"]()
    arrs = {"w1s": ws[0], "w2s": ws[1], "w3s": ws[2], "wos": ws[3]}
    wouts = wx["sharded"](*[arrs[n] for n in wx["in_names"]], *wzeros)
    wfull = dict(zip(wx["out_names"], wouts))
    _CACHE["whost"] = ws
    _CACHE["wfull"] = wfull
    return wfull


def kernel(query, key, value, w1, w2, w3, w_out, _trace=False):
    out, ok = _kernel_once(query, key, value, w1, w2, w3, w_out)
    # The remote runtime very occasionally returns a stale/corrupt buffer.
    # Output row scales from a real run are all in (0, ~2e-3); a stale
    # (zero-initialized or garbage) buffer fails this. Retry once.
    if not ok:
        out, ok = _kernel_once(query, key, value, w1, w2, w3, w_out)
    return out


def _kernel_once(query, key, value, w1, w2, w3, w_out):
    import jax
    st = _get_state()
    put = jax.device_put
    pool = st["pool"]

    query = np.asarray(query)
    value = np.asarray(value)

    wfull = _weights_device(st, w1, w2, w3, w_out)

    pending = []
    for i, (c0, tc) in enumerate(CHUNKS):
        kx = st["k"][tc]
        shard = kx["shard"]
        tcv = tc + 2 * VH
        zeros_f = pool.submit(kx["mkzeros"])
        # quantize q chunk, then v chunk (+halo), interleaved with async puts
        qq, qs = _quant_rows(query[:, c0:c0 + tc])
        vlo, vhi = max(0, c0 - VH), min(T, c0 + tc + VH)
        vq_i, vs_i = _quant_rows(value[:, vlo:vhi])
        vq = np.zeros((B, tcv, F), np.int8)
        vs = np.ones((B, tcv), np.float32)
        o = vlo - (c0 - VH)
        vq[:, o:o + vhi - vlo] = vq_i
        vs[:, o:o + vhi - vlo] = vs_i
        blob = np.concatenate(
            [qq.reshape(B, tc * F), vq.reshape(B, tcv * F)],
            axis=1).reshape(-1)
        blob_f = pool.submit(put, blob, shard)
        scl = np.concatenate([qs, vs], axis=1).reshape(B * (tc + tcv), 1)
        scl_f = pool.submit(put, scl, shard)
        arrays = {"blob": blob_f.result(), "scl": scl_f.result(),
                  "w1f": wfull["w1f"], "w2f": wfull["w2f"],
                  "w3f": wfull["w3f"], "wof": wfull["wof"]}
        ins = [arrays[n] for n in kx["in_names"]]
        outs = kx["sharded"](*ins, *zeros_f.result())
        for o_ in outs:
            o_.copy_to_host_async()   # start D2H as soon as exec finishes
        pending.append(dict(zip(kx["out_names"], outs)))

    # ---- collect: dequantize int8 rows with their f32 scales
    final = np.empty((B, T, F), np.float32)
    ok = True
    for (c0, tc), outs in zip(CHUNKS, pending):
        oq = np.asarray(outs["oq"]).reshape(B, tc, F)
        os_ = np.asarray(outs["os"]).reshape(B, tc, 1)
        np.multiply(oq.astype(np.float32), os_, out=final[:, c0:c0 + tc])
        smax = os_.max()
        smin = os_.min()
        if not (np.isfinite(smax) and 0.0 < smin and smax < 0.1):
            ok = False
    return final, ok


# revision 18
# speedup vs baseline: 1.9637x; 1.1779x over previous
"""Trainium2 Bass kernel for LocalDenseSynthesizerAttention.

Data-parallel over batch B=8 -> 8 cores, one batch each. The axon tunnel
(~45MB/s, effectively half-duplex) dominates, so the design minimizes wire
bytes:
  - q and v shipped int8 with per-row (per-t) scales; scales are bf16-rounded
    on host so host and device use bit-identical values, then shipped f32
  - v ships with a 64-row halo folded into the same array (window is local,
    pad = 22), so no separate halo tensor or device-side halo assembly
  - output quantized to int8 with per-row scales ON DEVICE (row absmax ->
    126/absmax), downloaded as int8 + f32 scales, dequantized on host.
    Device rounding uses the +1.5*2^23 magic-number trick so the result is
    exactly np.rint regardless of the convert instruction's rounding mode.
  - projection weights shipped f32 ONCE (content-compared per call, reuses
    device-resident copies), AllGathered from 8-way shards on device, stored
    pre-transposed [128, KF, N] fp16 for the compute launches
  - device compute in fp16 (PE supports fp16 matmul) instead of bf16: the
    extra mantissa bits keep total rel-err at baseline level (~1.0e-2)
    despite int8 I/O
  - q+v packed into one int8 blob per chunk (fewer device_puts; each put has
    a large fixed cost), puts dispatched from a small thread pool
  - compute split into sequence chunks, one 8-core launch per chunk, so host
    quantization overlaps wire transfer

The local window C=45 weighted sum is computed as banded matmuls: the banded
matrix B[s,t'] = attn[t0+t',h,s-t'] is an affine strided view of a zero-padded
attn tensor in DRAM, loaded matmul-ready via XBAR transpose-DMA.

Self-contained: hardcodes shapes from the problem spec.
"""
import sys
sys.path.insert(0, '/opt/trn_rl_repo')
import numpy as np

import concourse.bass as bass
import concourse.mybir as mybir
import concourse.tile as tile
from concourse import bacc
from concourse import masks

T, F = 2048, 512
H, C, DK = 8, 45, 64
HC = H * C          # 360
W = 128             # padded attn width per head (covers s-t' in [-63,127])
S = 64              # t' band-block size
PADV = 22           # (C-1)//2
KF = F // 128       # 4 contraction chunks
B = 8               # total batches / cores
FSH = F // B        # 64 weight-shard rows per core

VH = 64             # v halo rows each side (>= PADV, keeps tiles 128-aligned)
VOFF = VH - PADV    # chunk-vpad[r] = v_logical[r + VOFF]
CHUNKS = [(0, 1536), (1536, 512)]
import os as _os
if _os.environ.get("KCHUNKS"):
    _ls = [int(x) for x in _os.environ["KCHUNKS"].split(",")]
    assert sum(_ls) == T
    CHUNKS = []
    _c = 0
    for _l in _ls:
        CHUNKS.append((_c, _l))
        _c += _l

F16 = mybir.dt.float16
F32 = mybir.dt.float32
I8 = mybir.dt.int8
U8 = mybir.dt.uint8
QD = 126.0          # int8 quant denominator (126 leaves headroom for the
                    # bf16 round-down of the scale: 126*1.002 < 126.5)
QD6 = 30.0          # int6 quant denominator for q (codes in [-30, 30])
MAGIC = 12582912.0  # 1.5 * 2^23: fp32 add rounds the value to nearest int
Q6 = _os.environ.get("KQ6", "1") == "1"   # q in packed 6-bit (else int8)
QROW = 384 if Q6 else 512                 # q bytes per row on the wire

_CACHE = {}


# The build functions are compiled from a synthetic filename so the
# source-location debug info embedded in the BIR (and thus the NEFF
# cache key) does not depend on where this file lives.
_BUILD_SRC = r'''
def _build_w():
    """Weights launch (first call only): AllGather 8-way f32 weight shards,
    convert to fp16 in the matmul-ready [128, KF, N] layout, store to
    device-resident DRAM outputs."""
    nc = bacc.Bacc("TRN2", target_bir_lowering=False, debug=False,
                   num_devices=B, disable_frame_to_traceback=True)
    w1s = nc.dram_tensor("w1s", (FSH, F), F32, kind="ExternalInput")
    w2s = nc.dram_tensor("w2s", (FSH, HC), F32, kind="ExternalInput")
    w3s = nc.dram_tensor("w3s", (FSH, F), F32, kind="ExternalInput")
    wos = nc.dram_tensor("wos", (FSH, F), F32, kind="ExternalInput")
    w1f = nc.dram_tensor("w1f", (128, KF * F), F16, kind="ExternalOutput")
    w2f = nc.dram_tensor("w2f", (128, KF * HC), F16, kind="ExternalOutput")
    w3f = nc.dram_tensor("w3f", (128, KF * F), F16, kind="ExternalOutput")
    wof = nc.dram_tensor("wof", (128, KF * F), F16, kind="ExternalOutput")
    groups = [list(range(B))]
    with tile.TileContext(nc) as tc:
        with tc.tile_pool(name="dram", bufs=1, space="DRAM") as dp, \
             tc.tile_pool(name="sb", bufs=2) as sp:
            for idx, (shard, out, n) in enumerate(
                    ((w1s, w1f, F), (w2s, w2f, HC),
                     (w3s, w3f, F), (wos, wof, F))):
                stage = dp.tile([FSH, n], F32, name=f"st{idx}")
                full = dp.tile([F, n], F32, name=f"fu{idx}")
                # collectives cannot read IO tensors: stage shards first
                nc.sync.dma_start(stage[:, :], shard[:, :])
                nc.gpsimd.collective_compute(
                    "AllGather", mybir.AluOpType.bypass, groups,
                    [stage[:, :]], [full[:, :]])
                sb32 = sp.tile([128, KF, n], F32, tag=f"sb32_{n}")
                nc.sync.dma_start(
                    sb32[:], full[:, :].rearrange("(ko p) n -> p ko n", p=128))
                sb16 = sp.tile([128, KF, n], F16, tag=f"sb16_{n}")
                nc.scalar.copy(sb16[:], sb32[:])
                nc.sync.dma_start(
                    out[:, :].rearrange("p (ko n) -> p ko n", ko=KF), sb16[:])
    nc.compile()
    return nc


def _build_k(TC):
    """Compute launch for one sequence chunk of TC rows."""
    TCV = TC + 2 * VH           # v rows incl halo
    NT = TC // 128              # t-tiles in the chunk
    NTV = TCV // 128            # v tiles incl halo
    NB = TC // S                # band blocks
    nc = bacc.Bacc("TRN2", target_bir_lowering=False, debug=False,
                   num_devices=B, disable_frame_to_traceback=True)
    # qb = q rows (TC x QROW bytes: int8 codes, or 6-bit codes packed as
    # [P0|P1|P2] planes of 128 bytes) then TC f32 row scales (raw bytes);
    # vb = v int8 rows (TCV,F) then TCV f32 row scales. Separate tensors so
    # the host can ship q while it is still quantizing v.
    qb = nc.dram_tensor("qb", (TC * QROW + 4 * TC,), I8, kind="ExternalInput")
    vb = nc.dram_tensor("vb", (TCV * F + 4 * TCV,), I8, kind="ExternalInput")
    w1f = nc.dram_tensor("w1f", (128, KF * F), F16, kind="ExternalInput")
    w2f = nc.dram_tensor("w2f", (128, KF * HC), F16, kind="ExternalInput")
    w3f = nc.dram_tensor("w3f", (128, KF * F), F16, kind="ExternalInput")
    wof = nc.dram_tensor("wof", (128, KF * F), F16, kind="ExternalInput")
    oq = nc.dram_tensor("oq", (TC, F), I8, kind="ExternalOutput")
    os_ = nc.dram_tensor("os", (TC, 1), F32, kind="ExternalOutput")

    with tile.TileContext(nc) as tc:
        with tc.tile_pool(name="wpool", bufs=1) as wp, \
             tc.tile_pool(name="inpool", bufs=1) as inp, \
             tc.tile_pool(name="persist", bufs=1) as pers, \
             tc.tile_pool(name="work", bufs=2) as wk, \
             tc.tile_pool(name="band", bufs=4) as bp, \
             tc.tile_pool(name="psmain", bufs=2, space="PSUM") as psm, \
             tc.tile_pool(name="psband", bufs=4, space="PSUM") as psb, \
             tc.tile_pool(name="pstp", bufs=2, space="PSUM") as ptp, \
             tc.tile_pool(name="drampool", bufs=1, space="DRAM") as dp:

            # ---- weights to SBUF, [128, KF, n] fp16 (partition = contraction)
            w1_t = wp.tile([128, KF, F], F16, tag="w1")
            nc.sync.dma_start(
                w1_t[:], w1f[:, :].rearrange("p (ko n) -> p ko n", ko=KF))
            w2_t = wp.tile([128, KF, HC], F16, tag="w2")
            nc.sync.dma_start(
                w2_t[:], w2f[:, :].rearrange("p (ko n) -> p ko n", ko=KF))
            w3_t = wp.tile([128, KF, F], F16, tag="w3")
            nc.sync.dma_start(
                w3_t[:], w3f[:, :].rearrange("p (ko n) -> p ko n", ko=KF))
            wo_t = wp.tile([128, KF, F], F16, tag="wo")
            nc.sync.dma_start(
                wo_t[:], wof[:, :].rearrange("p (ko n) -> p ko n", ko=KF))

            ident = pers.tile([128, 128], F16, tag="ident")
            masks.make_identity(nc, ident[:])

            # ---- dequantize q and v (t-major int8 -> fp16), PE-transpose to
            # f-major [128 f, KF, t]
            qT = inp.tile([128, KF, TC], F16, tag="qT")
            vT = inp.tile([128, KF, TCV], F16, tag="vT")

            def _scale_ap(src_t, nrow, nbytes, tt):
                return src_t[nrow * nbytes + 4 * tt * 128:
                             nrow * nbytes + 4 * (tt + 1) * 128] \
                    .bitcast(F32).rearrange("(p n) -> p n", n=1)

            def _transpose_in(dst, dq, tt):
                for fo in range(KF):
                    pst = ptp.tile([128, 128], F16, tag="tp")
                    nc.tensor.transpose(
                        pst[:], dq[:, fo * 128:(fo + 1) * 128], ident[:])
                    nc.scalar.copy(dst[:, fo, tt * 128:(tt + 1) * 128],
                                   pst[:])

            # ---- q tiles: unpack (if 6-bit), dequant, transpose
            for tt in range(NT):
                sq = wk.tile([128, 1], F32, tag="sq")
                nc.sync.dma_start(sq[:], _scale_ap(qb, TC, QROW, tt))
                dq = wk.tile([128, F], F16, tag="dq")
                if not Q6:
                    i8 = wk.tile([128, F], I8, tag="i8")
                    src = qb[tt * 128 * F:(tt + 1) * 128 * F]
                    nc.sync.dma_start(
                        i8[:], src.rearrange("(p n) -> p n", n=F))
                    nc.scalar.activation(dq[:], i8[:],
                                         mybir.ActivationFunctionType.Copy,
                                         scale=sq[:, :])
                else:
                    # 6-bit unpack, all in exact fp32 arithmetic. Bytes are
                    # planes P_j (j=0,1,2) of 128 cols: low 6 bits = biased
                    # code of f=4k+j; high 2 bits = bits [2j,2j+2) of the
                    # f=4k+3 code. All values are small integers, so fp32
                    # mul/add and the magic-add round are exact.
                    pk = wk.tile([128, QROW], U8, tag="pk")
                    src = qb[tt * 128 * QROW:(tt + 1) * 128 * QROW] \
                        .bitcast(U8)
                    nc.sync.dma_start(
                        pk[:], src.rearrange("(p n) -> p n", n=QROW))
                    pf = wk.tile([128, QROW], F32, tag="pf")
                    nc.scalar.copy(pf[:], pk[:])
                    # hi = floor(pf / 64) via round(pf/64 - 63/128)
                    hi = wk.tile([128, QROW], F32, tag="hi")
                    nc.vector.tensor_scalar(hi[:], pf[:], 1.0 / 64,
                                            -0.4921875,
                                            mybir.AluOpType.mult,
                                            mybir.AluOpType.add)
                    nc.vector.tensor_scalar(hi[:], hi[:], MAGIC, -MAGIC,
                                            mybir.AluOpType.add,
                                            mybir.AluOpType.add)
                    # low6 = pf - 64*hi
                    lo = wk.tile([128, QROW], F32, tag="lo")
                    nc.vector.tensor_scalar(lo[:], hi[:], -64.0, None,
                                            mybir.AluOpType.mult)
                    nc.vector.tensor_add(out=lo[:], in0=lo[:], in1=pf[:])
                    # assemble biased codes c[f]: f=4k+j <- lo plane j;
                    # f=4k+3 <- hi0 + 4*hi1 + 16*hi2
                    ct = wk.tile([128, F], F32, tag="ct")
                    c4 = ct[:].rearrange("p (k j) -> p k j", j=4)
                    for j in range(3):
                        nc.vector.tensor_copy(
                            out=c4[:, :, j], in_=lo[:, j * 128:(j + 1) * 128])
                    t1 = wk.tile([128, 128], F32, tag="t1")
                    nc.vector.tensor_scalar(t1[:], hi[:, 128:256], 4.0, None,
                                            mybir.AluOpType.mult)
                    nc.vector.tensor_add(out=t1[:], in0=t1[:],
                                         in1=hi[:, 0:128])
                    t2 = wk.tile([128, 128], F32, tag="t2")
                    nc.vector.tensor_scalar(t2[:], hi[:, 256:384], 16.0, None,
                                            mybir.AluOpType.mult)
                    nc.vector.tensor_add(out=c4[:, :, 3], in0=t1[:],
                                         in1=t2[:])
                    # dequant: (c - 32) * s = c*s + (-32*s)
                    nbias = wk.tile([128, 1], F32, tag="nbias")
                    nc.vector.tensor_scalar(nbias[:], sq[:], -32.0, None,
                                            mybir.AluOpType.mult)
                    nc.scalar.activation(dq[:], ct[:],
                                         mybir.ActivationFunctionType.Identity,
                                         bias=nbias[:, :], scale=sq[:, :])
                _transpose_in(qT, dq, tt)

            # ---- v tiles: int8 dequant, transpose
            for tt in range(NTV):
                i8 = wk.tile([128, F], I8, tag="i8")
                src = vb[tt * 128 * F:(tt + 1) * 128 * F]
                nc.sync.dma_start(
                    i8[:], src.rearrange("(p n) -> p n", n=F))
                sq = wk.tile([128, 1], F32, tag="sq")
                nc.sync.dma_start(sq[:], _scale_ap(vb, TCV, F, tt))
                dq = wk.tile([128, F], F16, tag="dq")
                nc.scalar.activation(dq[:], i8[:],
                                     mybir.ActivationFunctionType.Copy,
                                     scale=sq[:, :])
                _transpose_in(vT, dq, tt)

            # ---- DRAM scratch
            # vproj rows j = w3-projection of v_in row j; v rows outside the
            # sequence are int8 zeros (scale 1) and project to exact zeros
            vproj = dp.tile([TCV, F], F16)
            # apad: 1 guard row + TC data rows + 1 guard row, row = [8 x 128]
            apad = dp.tile([TC + 2, H * W], F16)

            # zero tile for apad guards
            z_t = pers.tile([128, H * W], F16, tag="zt")
            nc.any.memzero(z_t[:])
            nc.sync.dma_start(apad[0:1, :], z_t[0:1, :])
            nc.sync.dma_start(apad[TC + 1:TC + 2, :], z_t[0:1, :])

            # ---- persistent SBUF activations
            qrT = pers.tile([128, KF, TC], F16, tag="qrT")  # relu(q@w1) f-major
            xT = pers.tile([128, KF, TC], F16, tag="xT")    # band out, f-major

            # ================= Phase A: q-proj + relu (f-major out) ===========
            for fo in range(KF):
                for tt in range(TC // 512):
                    ps = psm.tile([128, 512], F32, tag="mm")
                    for k in range(KF):
                        nc.tensor.matmul(
                            ps[:], w1_t[:, k, fo * 128:(fo + 1) * 128],
                            qT[:, k, tt * 512:(tt + 1) * 512],
                            start=(k == 0), stop=(k == KF - 1))
                    nc.scalar.activation(qrT[:, fo, tt * 512:(tt + 1) * 512],
                                         ps[:],
                                         mybir.ActivationFunctionType.Relu)

            # ================= Phase C: v-proj (t-major out) -> vproj =========
            for tb in range(NTV):
                ps = psm.tile([128, 512], F32, tag="mm")
                for k in range(KF):
                    nc.tensor.matmul(
                        ps[:], vT[:, k, tb * 128:(tb + 1) * 128],
                        w3_t[:, k, :],
                        start=(k == 0), stop=(k == KF - 1))
                v_sb = wk.tile([128, F], F16, tag="vsb")
                nc.scalar.copy(v_sb[:], ps[:])
                nc.sync.dma_start(vproj[tb * 128:(tb + 1) * 128, :], v_sb[:])

            # ====== Phase B: s-proj (t-major) + softmax -> apad (padded) ======
            for tb in range(NT):
                ps = psm.tile([128, 512], F32, tag="mm")
                for k in range(KF):
                    nc.tensor.matmul(
                        ps[:, 0:HC], qrT[:, k, tb * 128:(tb + 1) * 128],
                        w2_t[:, k, :],
                        start=(k == 0), stop=(k == KF - 1))
                e_t = wk.tile([128, HC], F32, tag="et")
                nc.scalar.activation(e_t[:], ps[:, 0:HC],
                                     mybir.ActivationFunctionType.Exp)
                zs = wk.tile([128, H], F32, tag="zs")
                nc.vector.reduce_sum(zs[:],
                                     e_t[:].rearrange("p (h c) -> p h c", c=C),
                                     axis=mybir.AxisListType.X)
                rz = wk.tile([128, H], F32, tag="rz")
                nc.vector.reciprocal(rz[:], zs[:])
                ap_t = wk.tile([128, H * W], F16, tag="apad")
                if tb < 2:
                    # zero the pad region once per pool slot (bufs=2); the pad
                    # columns are never overwritten afterwards
                    nc.any.memzero(ap_t[:])
                nc.vector.tensor_mul(
                    out=ap_t[:].rearrange("p (h w) -> p h w", w=W)[:, :, 0:C],
                    in0=e_t[:].rearrange("p (h c) -> p h c", c=C),
                    in1=rz[:, :, None].to_broadcast((128, H, C)))
                nc.sync.dma_start(apad[1 + tb * 128:1 + (tb + 1) * 128, :],
                                  ap_t[:])

            # ================= Phase D: banded attention matmuls ==============
            # x[t', h*64+d] = sum_s vproj[VOFF+t0+s, h*64+d] * B_h[s, t']
            # B_h loaded via transpose-DMA of sheared apad view.
            apad_h = apad.tensor  # underlying DRAM handle
            apad_off = apad.offset if isinstance(apad.offset, int) else 0
            for g in range(NB // 4):    # groups of 4 band blocks = 256 t'
                pss = [psb.tile([128, 512], F32, tag="px", name=f"px{g}_{pi}")
                       for pi in range(4)]
                for j in range(4):
                    bi = g * 4 + j
                    t0 = S * bi
                    vsp = wk.tile([128, F], F16, tag="vsp")
                    nc.sync.dma_start(vsp[:],
                                      vproj[VOFF + t0:VOFF + t0 + 128, :])
                    for p in range(4):      # head pairs
                        for i in range(2):
                            h = 2 * p + i
                            b_t = bp.tile([W, S], F16, tag="bt")
                            src = bass.AP(
                                tensor=apad_h,
                                offset=apad_off + (1 + t0) * (H * W) + h * W,
                                ap=[[H * W - 1, S], [1, W]])
                            eng = nc.scalar if h % 2 else nc.sync
                            eng.dma_start_transpose(b_t[:], src)
                            # lhsT = v head-pair [128, 128]; valid out rows are
                            # [i*64:(i+1)*64]; the other half is garbage and
                            # ignored at copyback.
                            nc.tensor.matmul(
                                pss[p][:, j * 128 + i * 64:
                                       j * 128 + (i + 1) * 64],
                                vsp[:, p * 128:(p + 1) * 128], b_t[:],
                                start=True, stop=True)
                # copy valid quadrants -> xT (f-major): fold p rows 0:63 = head
                # 2p (cols i=0), rows 64:127 = head 2p+1 (cols i=1)
                for p in range(4):
                    ps3 = pss[p][:].rearrange("d (j i k) -> d j i k", j=4, i=2)
                    dst = xT[:, p, g * 256:(g + 1) * 256] \
                        .rearrange("d (j k) -> d j k", j=4)
                    nc.vector.tensor_copy(out=dst[0:64], in_=ps3[0:64, :, 0, :])
                    nc.vector.tensor_copy(out=dst[64:128],
                                          in_=ps3[64:128, :, 1, :])

            # ========= Phase E: out-proj + per-row int8 quantization ==========
            for tb in range(NT):
                ps = psm.tile([128, 512], F32, tag="mm")
                for k in range(KF):
                    nc.tensor.matmul(
                        ps[:], xT[:, k, tb * 128:(tb + 1) * 128],
                        wo_t[:, k, :],
                        start=(k == 0), stop=(k == KF - 1))
                am = wk.tile([128, 1], F32, tag="am")
                nc.vector.reduce_max(am[:], ps[:], axis=mybir.AxisListType.X,
                                     apply_absolute_value=True)
                rz = wk.tile([128, 1], F32, tag="orz")
                nc.vector.reciprocal(rz[:], am[:])
                rs = wk.tile([128, 1], F32, tag="ors")
                nc.vector.tensor_scalar_mul(rs[:], rz[:], QD)
                y = wk.tile([128, F], F32, tag="oy")
                nc.scalar.activation(y[:], ps[:],
                                     mybir.ActivationFunctionType.Copy,
                                     scale=rs[:, :])
                # round to nearest int (RNE) via magic add/sub, then convert:
                # the value is exactly integral so the convert's rounding
                # mode is irrelevant
                yr = wk.tile([128, F], F32, tag="oyr")
                nc.vector.tensor_scalar(yr[:], y[:], MAGIC, -MAGIC,
                                        mybir.AluOpType.add,
                                        mybir.AluOpType.add)
                oqt = wk.tile([128, F], I8, tag="oqt")
                nc.vector.tensor_copy(out=oqt[:], in_=yr[:])
                ost = wk.tile([128, 1], F32, tag="ost")
                nc.vector.tensor_scalar_mul(ost[:], am[:], 1.0 / QD)
                nc.sync.dma_start(oq[tb * 128:(tb + 1) * 128, :], oqt[:])
                nc.scalar.dma_start(os_[tb * 128:(tb + 1) * 128, :], ost[:])

    nc.compile()
    return nc
'''

exec(compile(_BUILD_SRC, "bass_build_k", "exec"), globals())


def _make_exec(nc, devices):
    """Cached jitted executable for one bass module; outputs come from
    donated on-device zero buffers (mkzeros)."""
    import jax
    import jax.numpy as jnp
    from jax.sharding import Mesh, PartitionSpec, NamedSharding
    from jax.experimental.shard_map import shard_map
    from concourse.bass2jax import _bass_exec_p, partition_id_tensor

    partition_name = (nc.partition_id_tensor.name
                      if nc.partition_id_tensor else None)
    in_names, out_names, out_avals = [], [], []
    for alloc in nc.m.functions[0].allocations:
        if not isinstance(alloc, mybir.MemoryLocationSet):
            continue
        if alloc.kind not in ("ExternalInput", "ExternalOutput"):
            continue
        name = alloc.memorylocations[0].name
        if alloc.kind == "ExternalInput":
            if name != partition_name:
                in_names.append(name)
        else:
            out_avals.append(jax.core.ShapedArray(
                tuple(alloc.tensor_shape), mybir.dt.np(alloc.dtype)))
            out_names.append(name)
    n_params, n_outs = len(in_names), len(out_avals)
    in_names_all = list(in_names) + list(out_names)
    if partition_name is not None:
        in_names_all.append(partition_name)

    def _body(*args):
        operands = list(args)
        if partition_name is not None:
            operands.append(partition_id_tensor())
        return tuple(_bass_exec_p.bind(
            *operands,
            out_avals=tuple(out_avals),
            in_names=tuple(in_names_all),
            out_names=tuple(out_names),
            lowering_input_output_aliases=(),
            sim_require_finite=True,
            sim_require_nnan=True,
            nc=nc))

    n = len(devices)
    mesh = Mesh(np.asarray(devices), ("core",))
    shard = NamedSharding(mesh, PartitionSpec("core"))
    n_args = n_params + n_outs
    donate = tuple(range(n_params, n_args))
    mkzeros = jax.jit(
        lambda: tuple(jnp.zeros((n * a.shape[0], *a.shape[1:]), a.dtype)
                      for a in out_avals),
        out_shardings=(shard,) * n_outs)
    in_specs = (PartitionSpec("core"),) * n_args
    out_specs = (PartitionSpec("core"),) * n_outs
    sharded = jax.jit(
        shard_map(_body, mesh=mesh, in_specs=in_specs, out_specs=out_specs,
                  check_rep=False),
        donate_argnums=donate, keep_unused=True)
    return {"sharded": sharded, "mkzeros": mkzeros, "in_names": in_names,
            "out_names": out_names, "shard": shard, "n": n}


def _get_state():
    if "state" in _CACHE:
        return _CACHE["state"]
    import jax
    from concurrent.futures import ThreadPoolExecutor
    from concourse.bass2jax import install_neuronx_cc_hook
    install_neuronx_cc_hook()
    devices = jax.devices()[:B]
    wexec = _make_exec(_build_w(), devices)
    kexecs = {}
    for _, tc in CHUNKS:
        if tc not in kexecs:
            kexecs[tc] = _make_exec(_build_k(tc), devices)
    state = {"w": wexec, "k": kexecs,
             "pool": ThreadPoolExecutor(max_workers=3)}
    _CACHE["state"] = state
    return state


def _to_bf16_f32(x32):
    """fp32 -> bf16 via round-half-up on the upper 16 bits, returned as
    exact f32 values (so host and device share bit-identical scales)."""
    x32 = np.ascontiguousarray(x32, np.float32)
    tmp = x32.view(np.uint32) + np.uint32(0x8000)
    np.bitwise_and(tmp, np.uint32(0xFFFF0000), out=tmp)
    return tmp.view(np.float32)


def _quant_into(x, codes, scales, ybuf):
    """Per-row symmetric int8 quantization of (B, n, F) fp32 into
    preallocated codes (int8) and scales (f32, bf16-exact) views."""
    n = x.shape[1]
    a = np.maximum(x.max(axis=-1), -x.min(axis=-1))
    np.maximum(a, 1e-30, out=a)
    s = _to_bf16_f32(a / QD)
    y = ybuf[:, :n]
    np.multiply(x, (1.0 / s)[..., None], out=y)
    np.rint(y, out=y)
    codes[...] = y          # values are exactly integral: cast is exact
    scales[...] = s


def _quant6_into(x, codes, scales, ybuf):
    """Per-row 6-bit quantization of (B, n, F) fp32, packed 4 codes -> 3
    bytes in the plane layout the device kernel unpacks."""
    n = x.shape[1]
    a = np.maximum(x.max(axis=-1), -x.min(axis=-1))
    np.maximum(a, 1e-30, out=a)
    s = _to_bf16_f32(a / QD6)
    y = ybuf[:, :n]
    np.multiply(x, (1.0 / s)[..., None], out=y)
    np.rint(y, out=y)
    y += 32.0                   # biased codes in [2, 62]
    c = y.astype(np.uint8).reshape(-1, n, 128, 4)
    b3 = c[..., 3]
    cu = codes.view(np.uint8)
    cu[:, :, 0:128] = c[..., 0] + ((b3 & 3) << 6)
    cu[:, :, 128:256] = c[..., 1] + (((b3 >> 2) & 3) << 6)
    cu[:, :, 256:384] = c[..., 2] + ((b3 >> 4) << 6)
    scales[...] = s


def _weights_device(st, w1, w2, w3, w_out):
    """Device-resident fp16 weights, re-uploaded only when contents change."""
    import jax
    ws = [np.ascontiguousarray(np.asarray(w), np.float32)
          for w in (w1, w2, w3, w_out)]
    cached = _CACHE.get("wfull")
    if cached is not None and all(
            np.array_equal(a, b) for a, b in zip(ws, _CACHE["whost"])):
        return cached
    wx = st["w"]
    wzeros = wx["mkzeros"]()
    arrs = {"w1s": ws[0], "w2s": ws[1], "w3s": ws[2], "wos": ws[3]}
    wouts = wx["sharded"](*[arrs[n] for n in wx["in_names"]], *wzeros)
    wfull = dict(zip(wx["out_names"], wouts))
    _CACHE["whost"] = ws
    _CACHE["wfull"] = wfull
    return wfull


def kernel(query, key, value, w1, w2, w3, w_out, _trace=False):
    out, ok = _kernel_once(query, key, value, w1, w2, w3, w_out)
    # The remote runtime very occasionally returns a stale/corrupt buffer.
    # Output row scales from a real run are all in (0, ~2e-3); a stale
    # (zero-initialized or garbage) buffer fails this. Retry once.
    if not ok:
        out, ok = _kernel_once(query, key, value, w1, w2, w3, w_out)
    return out


def _kernel_once(query, key, value, w1, w2, w3, w_out):
    import jax, os, time
    st = _get_state()
    put = jax.device_put
    pool = st["pool"]
    timing = os.environ.get("KTIMING")
    tt0 = time.perf_counter()
    lap = lambda tag: timing and print(
        f"  [{tag}] {time.perf_counter() - tt0:.3f}s", flush=True)

    query = np.asarray(query)
    value = np.asarray(value)

    wfull = _weights_device(st, w1, w2, w3, w_out)
    # scratch fp32 buffer shared by all quantizations (max rows = tc + 2*VH)
    maxr = max(tc for _, tc in CHUNKS) + 2 * VH
    ybuf = _CACHE.get("ybuf")
    if ybuf is None or ybuf.shape[1] < maxr:
        ybuf = _CACHE["ybuf"] = np.empty((B, maxr, F), np.float32)
    lap("weights")

    pending = []
    for i, (c0, tc) in enumerate(CHUNKS):
        kx = st["k"][tc]
        shard = kx["shard"]
        tcv = tc + 2 * VH
        zeros_f = pool.submit(kx["mkzeros"])
        # q blob: codes then f32 row scales as raw bytes; ship it while
        # v is still being quantized
        qblob = np.empty((B, tc * QROW + 4 * tc), np.int8)
        qcodes = qblob[:, :tc * QROW].reshape(B, tc, QROW)
        qscales = qblob[:, tc * QROW:].view(np.float32)
        if Q6:
            _quant6_into(query[:, c0:c0 + tc], qcodes, qscales, ybuf)
        else:
            _quant_into(query[:, c0:c0 + tc], qcodes, qscales, ybuf)
        qb_f = pool.submit(put, qblob.reshape(-1), shard)
        lap(f"qput{i}")
        vblob = np.empty((B, tcv * F + 4 * tcv), np.int8)
        vcodes = vblob[:, :tcv * F].reshape(B, tcv, F)
        vscales = vblob[:, tcv * F:].view(np.float32)
        vlo, vhi = max(0, c0 - VH), min(T, c0 + tc + VH)
        o = vlo - (c0 - VH)
        if o:
            vcodes[:, :o] = 0
            vscales[:, :o] = 1.0
        if o + vhi - vlo < tcv:
            vcodes[:, o + vhi - vlo:] = 0
            vscales[:, o + vhi - vlo:] = 1.0
        _quant_into(value[:, vlo:vhi], vcodes[:, o:o + vhi - vlo],
                    vscales[:, o:o + vhi - vlo], ybuf)
        vb_f = pool.submit(put, vblob.reshape(-1), shard)
        lap(f"vput{i}")
        arrays = {"qb": qb_f.result(), "vb": vb_f.result(),
                  "w1f": wfull["w1f"], "w2f": wfull["w2f"],
                  "w3f": wfull["w3f"], "wof": wfull["wof"]}
        ins = [arrays[n] for n in kx["in_names"]]
        outs = kx["sharded"](*ins, *zeros_f.result())
        for o_ in outs:
            o_.copy_to_host_async()   # start D2H as soon as exec finishes
        lap(f"launch{i}")
        pending.append(dict(zip(kx["out_names"], outs)))

    # ---- collect: dequantize int8 rows with their f32 scales
    final = np.empty((B, T, F), np.float32)
    ok = True
    for ci, ((c0, tc), outs) in enumerate(zip(CHUNKS, pending)):
        oq = np.asarray(outs["oq"]).reshape(B, tc, F)
        os_ = np.asarray(outs["os"]).reshape(B, tc, 1)
        lap(f"fetch{ci}")
        np.multiply(oq, os_, out=final[:, c0:c0 + tc])
        smax = os_.max()
        smin = os_.min()
        if not (np.isfinite(smax) and 0.0 < smin and smax < 0.1):
            ok = False
    lap("dequant")
    return final, ok


# revision 24
# speedup vs baseline: 2.1041x; 1.0715x over previous
"""Trainium2 Bass kernel for LocalDenseSynthesizerAttention.

Data-parallel over batch B=8 -> 8 cores, one batch each. The axon tunnel
(~45MB/s, effectively half-duplex) dominates, so the design minimizes wire
bytes:
  - q and v shipped int8 with per-row (per-t) scales; scales are bf16-rounded
    on host so host and device use bit-identical values, then shipped f32
  - v ships with a 64-row halo folded into the same array (window is local,
    pad = 22), so no separate halo tensor or device-side halo assembly
  - output quantized to int8 with per-row scales ON DEVICE (row absmax ->
    126/absmax), downloaded as int8 + f32 scales, dequantized on host.
    Device rounding uses the +1.5*2^23 magic-number trick so the result is
    exactly np.rint regardless of the convert instruction's rounding mode.
  - projection weights shipped f32 ONCE (content-compared per call, reuses
    device-resident copies), AllGathered from 8-way shards on device, stored
    pre-transposed [128, KF, N] fp16 for the compute launches
  - device compute in fp16 (PE supports fp16 matmul) instead of bf16: the
    extra mantissa bits keep total rel-err at baseline level (~1.0e-2)
    despite int8 I/O
  - q+v packed into one int8 blob per chunk (fewer device_puts; each put has
    a large fixed cost), puts dispatched from a small thread pool
  - compute split into sequence chunks, one 8-core launch per chunk, so host
    quantization overlaps wire transfer

The local window C=45 weighted sum is computed as banded matmuls: the banded
matrix B[s,t'] = attn[t0+t',h,s-t'] is an affine strided view of a zero-padded
attn tensor in DRAM, loaded matmul-ready via XBAR transpose-DMA.

Self-contained: hardcodes shapes from the problem spec.
"""
import sys
sys.path.insert(0, '/opt/trn_rl_repo')
import numpy as np

import concourse.bass as bass
import concourse.mybir as mybir
import concourse.tile as tile
from concourse import bacc
from concourse import masks

T, F = 2048, 512
H, C, DK = 8, 45, 64
HC = H * C          # 360
W = 128             # padded attn width per head (covers s-t' in [-63,127])
S = 64              # t' band-block size
PADV = 22           # (C-1)//2
KF = F // 128       # 4 contraction chunks
B = 8               # total batches / cores
FSH = F // B        # 64 weight-shard rows per core

VH = 64             # v halo rows each side (>= PADV, keeps tiles 128-aligned)
VOFF = VH - PADV    # chunk-vpad[r] = v_logical[r + VOFF]
CHUNKS = [(0, 512), (512, 512), (1024, 512), (1536, 512)]
import os as _os
if _os.environ.get("KCHUNKS"):
    _ls = [int(x) for x in _os.environ["KCHUNKS"].split(",")]
    assert sum(_ls) == T
    CHUNKS = []
    _c = 0
    for _l in _ls:
        CHUNKS.append((_c, _l))
        _c += _l

F16 = mybir.dt.float16
F32 = mybir.dt.float32
I8 = mybir.dt.int8
U8 = mybir.dt.uint8
QD = 126.0          # int8 quant denominator (126 leaves headroom for the
                    # bf16 round-down of the scale: 126*1.002 < 126.5)
QD6 = 30.0          # int6 quant denominator for q (codes in [-30, 30])
MAGIC = 12582912.0  # 1.5 * 2^23: fp32 add rounds the value to nearest int
Q6 = _os.environ.get("KQ6", "1") == "1"   # q in packed 6-bit (else int8)
QROW = 384 if Q6 else 512                 # q bytes per row on the wire

_CACHE = {}


# The build functions are compiled from a synthetic filename so the
# source-location debug info embedded in the BIR (and thus the NEFF
# cache key) does not depend on where this file lives.
_BUILD_SRC = r'''
def _build_w():
    """Weights launch (first call only): AllGather 8-way f32 weight shards,
    convert to fp16 in the matmul-ready [128, KF, N] layout, store to
    device-resident DRAM outputs."""
    nc = bacc.Bacc("TRN2", target_bir_lowering=False, debug=False,
                   num_devices=B, disable_frame_to_traceback=True)
    w1s = nc.dram_tensor("w1s", (FSH, F), F32, kind="ExternalInput")
    w2s = nc.dram_tensor("w2s", (FSH, HC), F32, kind="ExternalInput")
    w3s = nc.dram_tensor("w3s", (FSH, F), F32, kind="ExternalInput")
    wos = nc.dram_tensor("wos", (FSH, F), F32, kind="ExternalInput")
    w1f = nc.dram_tensor("w1f", (128, KF * F), F16, kind="ExternalOutput")
    w2f = nc.dram_tensor("w2f", (128, KF * HC), F16, kind="ExternalOutput")
    w3f = nc.dram_tensor("w3f", (128, KF * F), F16, kind="ExternalOutput")
    wof = nc.dram_tensor("wof", (128, KF * F), F16, kind="ExternalOutput")
    groups = [list(range(B))]
    with tile.TileContext(nc) as tc:
        with tc.tile_pool(name="dram", bufs=1, space="DRAM") as dp, \
             tc.tile_pool(name="sb", bufs=2) as sp:
            for idx, (shard, out, n) in enumerate(
                    ((w1s, w1f, F), (w2s, w2f, HC),
                     (w3s, w3f, F), (wos, wof, F))):
                stage = dp.tile([FSH, n], F32, name=f"st{idx}")
                full = dp.tile([F, n], F32, name=f"fu{idx}")
                # collectives cannot read IO tensors: stage shards first
                nc.sync.dma_start(stage[:, :], shard[:, :])
                nc.gpsimd.collective_compute(
                    "AllGather", mybir.AluOpType.bypass, groups,
                    [stage[:, :]], [full[:, :]])
                sb32 = sp.tile([128, KF, n], F32, tag=f"sb32_{n}")
                nc.sync.dma_start(
                    sb32[:], full[:, :].rearrange("(ko p) n -> p ko n", p=128))
                sb16 = sp.tile([128, KF, n], F16, tag=f"sb16_{n}")
                nc.scalar.copy(sb16[:], sb32[:])
                nc.sync.dma_start(
                    out[:, :].rearrange("p (ko n) -> p ko n", ko=KF), sb16[:])
    nc.compile()
    return nc


def _build_k(TC, C0):
    """Compute launch for one sequence chunk of TC rows starting at C0."""
    TCV = TC + 2 * VH           # v rows incl halo
    NT = TC // 128              # t-tiles in the chunk
    NTV = TCV // 128            # v tiles incl halo
    NB = TC // S                # band blocks
    NR = T + 2 * VH             # rows of the shared padded v tensor
    nc = bacc.Bacc("TRN2", target_bir_lowering=False, debug=False,
                   num_devices=B, disable_frame_to_traceback=True)
    # qb = q rows (TC x QROW bytes: int8 codes, or 6-bit codes packed as
    # [P0|P1|P2] planes of 128 bytes) then TC f32 row scales (raw bytes);
    # vb = the WHOLE zero-padded v (shared by all chunk launches): NR int8
    # rows then NR f32 row scales; this chunk reads rows [C0, C0+TCV).
    qb = nc.dram_tensor("qb", (TC * QROW + 4 * TC,), I8, kind="ExternalInput")
    vb = nc.dram_tensor("vb", (NR * F + 4 * NR,), I8, kind="ExternalInput")
    w1f = nc.dram_tensor("w1f", (128, KF * F), F16, kind="ExternalInput")
    w2f = nc.dram_tensor("w2f", (128, KF * HC), F16, kind="ExternalInput")
    w3f = nc.dram_tensor("w3f", (128, KF * F), F16, kind="ExternalInput")
    wof = nc.dram_tensor("wof", (128, KF * F), F16, kind="ExternalInput")
    oq = nc.dram_tensor("oq", (TC, F), I8, kind="ExternalOutput")
    os_ = nc.dram_tensor("os", (TC, 1), F32, kind="ExternalOutput")

    with tile.TileContext(nc) as tc:
        with tc.tile_pool(name="wpool", bufs=1) as wp, \
             tc.tile_pool(name="inpool", bufs=1) as inp, \
             tc.tile_pool(name="persist", bufs=1) as pers, \
             tc.tile_pool(name="work", bufs=2) as wk, \
             tc.tile_pool(name="band", bufs=4) as bp, \
             tc.tile_pool(name="psmain", bufs=2, space="PSUM") as psm, \
             tc.tile_pool(name="psband", bufs=4, space="PSUM") as psb, \
             tc.tile_pool(name="pstp", bufs=2, space="PSUM") as ptp, \
             tc.tile_pool(name="drampool", bufs=1, space="DRAM") as dp:

            # ---- weights to SBUF, [128, KF, n] fp16 (partition = contraction)
            w1_t = wp.tile([128, KF, F], F16, tag="w1")
            nc.sync.dma_start(
                w1_t[:], w1f[:, :].rearrange("p (ko n) -> p ko n", ko=KF))
            w2_t = wp.tile([128, KF, HC], F16, tag="w2")
            nc.sync.dma_start(
                w2_t[:], w2f[:, :].rearrange("p (ko n) -> p ko n", ko=KF))
            w3_t = wp.tile([128, KF, F], F16, tag="w3")
            nc.sync.dma_start(
                w3_t[:], w3f[:, :].rearrange("p (ko n) -> p ko n", ko=KF))
            wo_t = wp.tile([128, KF, F], F16, tag="wo")
            nc.sync.dma_start(
                wo_t[:], wof[:, :].rearrange("p (ko n) -> p ko n", ko=KF))

            ident = pers.tile([128, 128], F16, tag="ident")
            masks.make_identity(nc, ident[:])

            # ---- dequantize q and v (t-major int8 -> fp16), PE-transpose to
            # f-major [128 f, KF, t]
            qT = inp.tile([128, KF, TC], F16, tag="qT")
            vT = inp.tile([128, KF, TCV], F16, tag="vT")

            def _scale_ap(src_t, nrow, nbytes, tt):
                return src_t[nrow * nbytes + 4 * tt * 128:
                             nrow * nbytes + 4 * (tt + 1) * 128] \
                    .bitcast(F32).rearrange("(p n) -> p n", n=1)

            def _transpose_in(dst, dq, tt):
                for fo in range(KF):
                    pst = ptp.tile([128, 128], F16, tag="tp")
                    nc.tensor.transpose(
                        pst[:], dq[:, fo * 128:(fo + 1) * 128], ident[:])
                    nc.scalar.copy(dst[:, fo, tt * 128:(tt + 1) * 128],
                                   pst[:])

            # ---- q tiles: unpack (if 6-bit), dequant, transpose
            for tt in range(NT):
                sq = wk.tile([128, 1], F32, tag="sq")
                nc.sync.dma_start(sq[:], _scale_ap(qb, TC, QROW, tt))
                dq = wk.tile([128, F], F16, tag="dq")
                if not Q6:
                    i8 = wk.tile([128, F], I8, tag="i8")
                    src = qb[tt * 128 * F:(tt + 1) * 128 * F]
                    nc.sync.dma_start(
                        i8[:], src.rearrange("(p n) -> p n", n=F))
                    nc.scalar.activation(dq[:], i8[:],
                                         mybir.ActivationFunctionType.Copy,
                                         scale=sq[:, :])
                else:
                    # 6-bit unpack, all in exact fp32 arithmetic. Bytes are
                    # planes P_j (j=0,1,2) of 128 cols: low 6 bits = biased
                    # code of f=4k+j; high 2 bits = bits [2j,2j+2) of the
                    # f=4k+3 code. All values are small integers, so fp32
                    # mul/add and the magic-add round are exact.
                    pk = wk.tile([128, QROW], U8, tag="pk")
                    src = qb[tt * 128 * QROW:(tt + 1) * 128 * QROW] \
                        .bitcast(U8)
                    nc.sync.dma_start(
                        pk[:], src.rearrange("(p n) -> p n", n=QROW))
                    pf = wk.tile([128, QROW], F32, tag="pf")
                    nc.scalar.copy(pf[:], pk[:])
                    # hi = floor(pf / 64) via round(pf/64 - 63/128)
                    hi = wk.tile([128, QROW], F32, tag="hi")
                    nc.vector.tensor_scalar(hi[:], pf[:], 1.0 / 64,
                                            -0.4921875,
                                            mybir.AluOpType.mult,
                                            mybir.AluOpType.add)
                    nc.vector.tensor_scalar(hi[:], hi[:], MAGIC, -MAGIC,
                                            mybir.AluOpType.add,
                                            mybir.AluOpType.add)
                    # low6 = pf - 64*hi
                    lo = wk.tile([128, QROW], F32, tag="lo")
                    nc.vector.tensor_scalar(lo[:], hi[:], -64.0, None,
                                            mybir.AluOpType.mult)
                    nc.vector.tensor_add(out=lo[:], in0=lo[:], in1=pf[:])
                    # assemble biased codes c[f]: f=4k+j <- lo plane j;
                    # f=4k+3 <- hi0 + 4*hi1 + 16*hi2
                    ct = wk.tile([128, F], F32, tag="ct")
                    c4 = ct[:].rearrange("p (k j) -> p k j", j=4)
                    for j in range(3):
                        nc.vector.tensor_copy(
                            out=c4[:, :, j], in_=lo[:, j * 128:(j + 1) * 128])
                    t1 = wk.tile([128, 128], F32, tag="t1")
                    nc.vector.tensor_scalar(t1[:], hi[:, 128:256], 4.0, None,
                                            mybir.AluOpType.mult)
                    nc.vector.tensor_add(out=t1[:], in0=t1[:],
                                         in1=hi[:, 0:128])
                    t2 = wk.tile([128, 128], F32, tag="t2")
                    nc.vector.tensor_scalar(t2[:], hi[:, 256:384], 16.0, None,
                                            mybir.AluOpType.mult)
                    nc.vector.tensor_add(out=c4[:, :, 3], in0=t1[:],
                                         in1=t2[:])
                    # dequant: (c - 32) * s = c*s + (-32*s)
                    nbias = wk.tile([128, 1], F32, tag="nbias")
                    nc.vector.tensor_scalar(nbias[:], sq[:], -32.0, None,
                                            mybir.AluOpType.mult)
                    nc.scalar.activation(dq[:], ct[:],
                                         mybir.ActivationFunctionType.Identity,
                                         bias=nbias[:, :], scale=sq[:, :])
                _transpose_in(qT, dq, tt)

            # ---- v tiles: int8 dequant, transpose (rows C0+tt*128 of vb)
            for tt in range(NTV):
                r0 = C0 + tt * 128
                i8 = wk.tile([128, F], I8, tag="i8")
                src = vb[r0 * F:(r0 + 128) * F]
                nc.sync.dma_start(
                    i8[:], src.rearrange("(p n) -> p n", n=F))
                sq = wk.tile([128, 1], F32, tag="sq")
                ssrc = vb[NR * F + 4 * r0:NR * F + 4 * (r0 + 128)] \
                    .bitcast(F32).rearrange("(p n) -> p n", n=1)
                nc.sync.dma_start(sq[:], ssrc)
                dq = wk.tile([128, F], F16, tag="dq")
                nc.scalar.activation(dq[:], i8[:],
                                     mybir.ActivationFunctionType.Copy,
                                     scale=sq[:, :])
                _transpose_in(vT, dq, tt)

            # ---- DRAM scratch
            # vproj rows j = w3-projection of v_in row j; v rows outside the
            # sequence are int8 zeros (scale 1) and project to exact zeros
            vproj = dp.tile([TCV, F], F16)
            # apad: 1 guard row + TC data rows + 1 guard row, row = [8 x 128]
            apad = dp.tile([TC + 2, H * W], F16)

            # zero tile for apad guards
            z_t = pers.tile([128, H * W], F16, tag="zt")
            nc.any.memzero(z_t[:])
            nc.sync.dma_start(apad[0:1, :], z_t[0:1, :])
            nc.sync.dma_start(apad[TC + 1:TC + 2, :], z_t[0:1, :])

            # ---- persistent SBUF activations
            qrT = pers.tile([128, KF, TC], F16, tag="qrT")  # relu(q@w1) f-major
            xT = pers.tile([128, KF, TC], F16, tag="xT")    # band out, f-major

            # ================= Phase A: q-proj + relu (f-major out) ===========
            for fo in range(KF):
                for tt in range(TC // 512):
                    ps = psm.tile([128, 512], F32, tag="mm")
                    for k in range(KF):
                        nc.tensor.matmul(
                            ps[:], w1_t[:, k, fo * 128:(fo + 1) * 128],
                            qT[:, k, tt * 512:(tt + 1) * 512],
                            start=(k == 0), stop=(k == KF - 1))
                    nc.scalar.activation(qrT[:, fo, tt * 512:(tt + 1) * 512],
                                         ps[:],
                                         mybir.ActivationFunctionType.Relu)

            # ================= Phase C: v-proj (t-major out) -> vproj =========
            for tb in range(NTV):
                ps = psm.tile([128, 512], F32, tag="mm")
                for k in range(KF):
                    nc.tensor.matmul(
                        ps[:], vT[:, k, tb * 128:(tb + 1) * 128],
                        w3_t[:, k, :],
                        start=(k == 0), stop=(k == KF - 1))
                v_sb = wk.tile([128, F], F16, tag="vsb")
                nc.scalar.copy(v_sb[:], ps[:])
                nc.sync.dma_start(vproj[tb * 128:(tb + 1) * 128, :], v_sb[:])

            # ====== Phase B: s-proj (t-major) + softmax -> apad (padded) ======
            for tb in range(NT):
                ps = psm.tile([128, 512], F32, tag="mm")
                for k in range(KF):
                    nc.tensor.matmul(
                        ps[:, 0:HC], qrT[:, k, tb * 128:(tb + 1) * 128],
                        w2_t[:, k, :],
                        start=(k == 0), stop=(k == KF - 1))
                e_t = wk.tile([128, HC], F32, tag="et")
                nc.scalar.activation(e_t[:], ps[:, 0:HC],
                                     mybir.ActivationFunctionType.Exp)
                zs = wk.tile([128, H], F32, tag="zs")
                nc.vector.reduce_sum(zs[:],
                                     e_t[:].rearrange("p (h c) -> p h c", c=C),
                                     axis=mybir.AxisListType.X)
                rz = wk.tile([128, H], F32, tag="rz")
                nc.vector.reciprocal(rz[:], zs[:])
                ap_t = wk.tile([128, H * W], F16, tag="apad")
                if tb < 2:
                    # zero the pad region once per pool slot (bufs=2); the pad
                    # columns are never overwritten afterwards
                    nc.any.memzero(ap_t[:])
                nc.vector.tensor_mul(
                    out=ap_t[:].rearrange("p (h w) -> p h w", w=W)[:, :, 0:C],
                    in0=e_t[:].rearrange("p (h c) -> p h c", c=C),
                    in1=rz[:, :, None].to_broadcast((128, H, C)))
                nc.sync.dma_start(apad[1 + tb * 128:1 + (tb + 1) * 128, :],
                                  ap_t[:])

            # ================= Phase D: banded attention matmuls ==============
            # x[t', h*64+d] = sum_s vproj[VOFF+t0+s, h*64+d] * B_h[s, t']
            # B_h loaded via transpose-DMA of sheared apad view.
            apad_h = apad.tensor  # underlying DRAM handle
            apad_off = apad.offset if isinstance(apad.offset, int) else 0
            for g in range(NB // 4):    # groups of 4 band blocks = 256 t'
                pss = [psb.tile([128, 512], F32, tag="px", name=f"px{g}_{pi}")
                       for pi in range(4)]
                for j in range(4):
                    bi = g * 4 + j
                    t0 = S * bi
                    vsp = wk.tile([128, F], F16, tag="vsp")
                    nc.sync.dma_start(vsp[:],
                                      vproj[VOFF + t0:VOFF + t0 + 128, :])
                    for p in range(4):      # head pairs
                        for i in range(2):
                            h = 2 * p + i
                            b_t = bp.tile([W, S], F16, tag="bt")
                            src = bass.AP(
                                tensor=apad_h,
                                offset=apad_off + (1 + t0) * (H * W) + h * W,
                                ap=[[H * W - 1, S], [1, W]])
                            eng = nc.scalar if h % 2 else nc.sync
                            eng.dma_start_transpose(b_t[:], src)
                            # lhsT = v head-pair [128, 128]; valid out rows are
                            # [i*64:(i+1)*64]; the other half is garbage and
                            # ignored at copyback.
                            nc.tensor.matmul(
                                pss[p][:, j * 128 + i * 64:
                                       j * 128 + (i + 1) * 64],
                                vsp[:, p * 128:(p + 1) * 128], b_t[:],
                                start=True, stop=True)
                # copy valid quadrants -> xT (f-major): fold p rows 0:63 = head
                # 2p (cols i=0), rows 64:127 = head 2p+1 (cols i=1)
                for p in range(4):
                    ps3 = pss[p][:].rearrange("d (j i k) -> d j i k", j=4, i=2)
                    dst = xT[:, p, g * 256:(g + 1) * 256] \
                        .rearrange("d (j k) -> d j k", j=4)
                    nc.vector.tensor_copy(out=dst[0:64], in_=ps3[0:64, :, 0, :])
                    nc.vector.tensor_copy(out=dst[64:128],
                                          in_=ps3[64:128, :, 1, :])

            # ========= Phase E: out-proj + per-row int8 quantization ==========
            for tb in range(NT):
                ps = psm.tile([128, 512], F32, tag="mm")
                for k in range(KF):
                    nc.tensor.matmul(
                        ps[:], xT[:, k, tb * 128:(tb + 1) * 128],
                        wo_t[:, k, :],
                        start=(k == 0), stop=(k == KF - 1))
                am = wk.tile([128, 1], F32, tag="am")
                nc.vector.reduce_max(am[:], ps[:], axis=mybir.AxisListType.X,
                                     apply_absolute_value=True)
                rz = wk.tile([128, 1], F32, tag="orz")
                nc.vector.reciprocal(rz[:], am[:])
                rs = wk.tile([128, 1], F32, tag="ors")
                nc.vector.tensor_scalar_mul(rs[:], rz[:], QD)
                y = wk.tile([128, F], F32, tag="oy")
                nc.scalar.activation(y[:], ps[:],
                                     mybir.ActivationFunctionType.Copy,
                                     scale=rs[:, :])
                # round to nearest int (RNE) via magic add/sub, then convert:
                # the value is exactly integral so the convert's rounding
                # mode is irrelevant
                yr = wk.tile([128, F], F32, tag="oyr")
                nc.vector.tensor_scalar(yr[:], y[:], MAGIC, -MAGIC,
                                        mybir.AluOpType.add,
                                        mybir.AluOpType.add)
                oqt = wk.tile([128, F], I8, tag="oqt")
                nc.vector.tensor_copy(out=oqt[:], in_=yr[:])
                ost = wk.tile([128, 1], F32, tag="ost")
                nc.vector.tensor_scalar_mul(ost[:], am[:], 1.0 / QD)
                nc.sync.dma_start(oq[tb * 128:(tb + 1) * 128, :], oqt[:])
                nc.scalar.dma_start(os_[tb * 128:(tb + 1) * 128, :], ost[:])

    nc.compile()
    return nc
'''

exec(compile(_BUILD_SRC, "bass_build_k", "exec"), globals())


def _make_exec(nc, devices):
    """Cached jitted executable for one bass module; outputs come from
    donated on-device zero buffers (mkzeros)."""
    import jax
    import jax.numpy as jnp
    from jax.sharding import Mesh, PartitionSpec, NamedSharding
    from jax.experimental.shard_map import shard_map
    from concourse.bass2jax import _bass_exec_p, partition_id_tensor

    partition_name = (nc.partition_id_tensor.name
                      if nc.partition_id_tensor else None)
    in_names, out_names, out_avals = [], [], []
    for alloc in nc.m.functions[0].allocations:
        if not isinstance(alloc, mybir.MemoryLocationSet):
            continue
        if alloc.kind not in ("ExternalInput", "ExternalOutput"):
            continue
        name = alloc.memorylocations[0].name
        if alloc.kind == "ExternalInput":
            if name != partition_name:
                in_names.append(name)
        else:
            out_avals.append(jax.core.ShapedArray(
                tuple(alloc.tensor_shape), mybir.dt.np(alloc.dtype)))
            out_names.append(name)
    n_params, n_outs = len(in_names), len(out_avals)
    in_names_all = list(in_names) + list(out_names)
    if partition_name is not None:
        in_names_all.append(partition_name)

    def _body(*args):
        operands = list(args)
        if partition_name is not None:
            operands.append(partition_id_tensor())
        return tuple(_bass_exec_p.bind(
            *operands,
            out_avals=tuple(out_avals),
            in_names=tuple(in_names_all),
            out_names=tuple(out_names),
            lowering_input_output_aliases=(),
            sim_require_finite=True,
            sim_require_nnan=True,
            nc=nc))

    n = len(devices)
    mesh = Mesh(np.asarray(devices), ("core",))
    shard = NamedSharding(mesh, PartitionSpec("core"))
    n_args = n_params + n_outs
    donate = tuple(range(n_params, n_args))
    mkzeros = jax.jit(
        lambda: tuple(jnp.zeros((n * a.shape[0], *a.shape[1:]), a.dtype)
                      for a in out_avals),
        out_shardings=(shard,) * n_outs)
    in_specs = (PartitionSpec("core"),) * n_args
    out_specs = (PartitionSpec("core"),) * n_outs
    sharded = jax.jit(
        shard_map(_body, mesh=mesh, in_specs=in_specs, out_specs=out_specs,
                  check_rep=False),
        donate_argnums=donate, keep_unused=True)
    return {"sharded": sharded, "mkzeros": mkzeros, "in_names": in_names,
            "out_names": out_names, "shard": shard, "n": n}


def _get_state():
    if "state" in _CACHE:
        return _CACHE["state"]
    import jax
    from concurrent.futures import ThreadPoolExecutor
    from concourse.bass2jax import install_neuronx_cc_hook
    install_neuronx_cc_hook()
    devices = jax.devices()[:B]
    wexec = _make_exec(_build_w(), devices)
    kexecs = {}
    for c0, tc in CHUNKS:
        kexecs[(tc, c0)] = _make_exec(_build_k(tc, c0), devices)
    state = {"w": wexec, "k": kexecs,
             "pool": ThreadPoolExecutor(max_workers=3)}
    _CACHE["state"] = state
    return state


def _to_bf16_f32(x32):
    """fp32 -> bf16 via round-half-up on the upper 16 bits, returned as
    exact f32 values (so host and device share bit-identical scales)."""
    x32 = np.ascontiguousarray(x32, np.float32)
    tmp = x32.view(np.uint32) + np.uint32(0x8000)
    np.bitwise_and(tmp, np.uint32(0xFFFF0000), out=tmp)
    return tmp.view(np.float32)


def _quant_into(x, codes, scales, ybuf):
    """Per-row symmetric int8 quantization of (B, n, F) fp32 into
    preallocated codes (int8) and scales (f32, bf16-exact) views."""
    n = x.shape[1]
    a = np.maximum(x.max(axis=-1), -x.min(axis=-1))
    np.maximum(a, 1e-30, out=a)
    s = _to_bf16_f32(a / QD)
    y = ybuf[:, :n]
    np.multiply(x, (1.0 / s)[..., None], out=y)
    np.rint(y, out=y)
    codes[...] = y          # values are exactly integral: cast is exact
    scales[...] = s


def _quant6_into(x, codes, scales, ybuf):
    """Per-row 6-bit quantization of (B, n, F) fp32, packed 4 codes -> 3
    bytes in the plane layout the device kernel unpacks."""
    n = x.shape[1]
    a = np.maximum(x.max(axis=-1), -x.min(axis=-1))
    np.maximum(a, 1e-30, out=a)
    s = _to_bf16_f32(a / QD6)
    y = ybuf[:, :n]
    np.multiply(x, (1.0 / s)[..., None], out=y)
    np.rint(y, out=y)
    y += 32.0                   # biased codes in [2, 62]
    c = y.astype(np.uint8).reshape(-1, n, 128, 4)
    b3 = c[..., 3]
    cu = codes.view(np.uint8)
    cu[:, :, 0:128] = c[..., 0] + ((b3 & 3) << 6)
    cu[:, :, 128:256] = c[..., 1] + (((b3 >> 2) & 3) << 6)
    cu[:, :, 256:384] = c[..., 2] + ((b3 >> 4) << 6)
    scales[...] = s


def _weights_device(st, w1, w2, w3, w_out):
    """Device-resident fp16 weights, re-uploaded only when contents change."""
    import jax
    ws = [np.ascontiguousarray(np.asarray(w), np.float32)
          for w in (w1, w2, w3, w_out)]
    cached = _CACHE.get("wfull")
    if cached is not None and all(
            np.array_equal(a, b) for a, b in zip(ws, _CACHE["whost"])):
        return cached
    wx = st["w"]
    wzeros = wx["mkzeros"]()
    arrs = {"w1s": ws[0], "w2s": ws[1], "w3s": ws[2], "wos": ws[3]}
    wouts = wx["sharded"](*[arrs[n] for n in wx["in_names"]], *wzeros)
    wfull = dict(zip(wx["out_names"], wouts))
    _CACHE["whost"] = ws
    _CACHE["wfull"] = wfull
    return wfull


def kernel(query, key, value, w1, w2, w3, w_out, _trace=False):
    out, ok = _kernel_once(query, key, value, w1, w2, w3, w_out)
    # The remote runtime very occasionally returns a stale/corrupt buffer.
    # Output row scales from a real run are all in (0, ~2e-3); a stale
    # (zero-initialized or garbage) buffer fails this. Retry once.
    if not ok:
        out, ok = _kernel_once(query, key, value, w1, w2, w3, w_out)
    return out


def _kernel_once(query, key, value, w1, w2, w3, w_out):
    import jax, os, time
    st = _get_state()
    put = jax.device_put
    pool = st["pool"]
    timing = os.environ.get("KTIMING")
    tt0 = time.perf_counter()
    lap = lambda tag: timing and print(
        f"  [{tag}] {time.perf_counter() - tt0:.3f}s", flush=True)

    query = np.asarray(query)
    value = np.asarray(value)

    wfull = _weights_device(st, w1, w2, w3, w_out)
    # scratch fp32 buffer shared by all quantizations (v uses all T rows)
    ybuf = _CACHE.get("ybuf")
    if ybuf is None:
        ybuf = _CACHE["ybuf"] = np.empty((B, T, F), np.float32)
    lap("weights")

    pending = []
    vb_f = None
    for i, (c0, tc) in enumerate(CHUNKS):
        kx = st["k"][(tc, c0)]
        shard = kx["shard"]
        zeros_f = pool.submit(kx["mkzeros"])
        # q blob: codes then f32 row scales as raw bytes; ship it while
        # v is still being quantized
        qblob = np.empty((B, tc * QROW + 4 * tc), np.int8)
        qcodes = qblob[:, :tc * QROW].reshape(B, tc, QROW)
        qscales = qblob[:, tc * QROW:].view(np.float32)
        if Q6:
            _quant6_into(query[:, c0:c0 + tc], qcodes, qscales, ybuf)
        else:
            _quant_into(query[:, c0:c0 + tc], qcodes, qscales, ybuf)
        qb_f = pool.submit(put, qblob.reshape(-1), shard)
        lap(f"qput{i}")
        if vb_f is None:
            # the whole zero-padded v ships once, shared by all launches
            nr = T + 2 * VH
            vblob = np.empty((B, nr * F + 4 * nr), np.int8)
            vcodes = vblob[:, :nr * F].reshape(B, nr, F)
            vscales = vblob[:, nr * F:].view(np.float32)
            vcodes[:, :VH] = 0
            vscales[:, :VH] = 1.0
            vcodes[:, VH + T:] = 0
            vscales[:, VH + T:] = 1.0
            _quant_into(value, vcodes[:, VH:VH + T],
                        vscales[:, VH:VH + T], ybuf)
            vb_f = pool.submit(put, vblob.reshape(-1), shard)
            lap("vput")
        arrays = {"qb": qb_f.result(), "vb": vb_f.result(),
                  "w1f": wfull["w1f"], "w2f": wfull["w2f"],
                  "w3f": wfull["w3f"], "wof": wfull["wof"]}
        ins = [arrays[n] for n in kx["in_names"]]
        outs = kx["sharded"](*ins, *zeros_f.result())
        for o_ in outs:
            o_.copy_to_host_async()   # start D2H as soon as exec finishes
        lap(f"launch{i}")
        pending.append(dict(zip(kx["out_names"], outs)))

    # ---- collect: dequantize int8 rows with their f32 scales
    final = np.empty((B, T, F), np.float32)
    ok = True
    for ci, ((c0, tc), outs) in enumerate(zip(CHUNKS, pending)):
        oq = np.asarray(outs["oq"]).reshape(B, tc, F)
        os_ = np.asarray(outs["os"]).reshape(B, tc, 1)
        lap(f"fetch{ci}")
        np.multiply(oq, os_, out=final[:, c0:c0 + tc])
        smax = os_.max()
        smin = os_.min()
        if not (np.isfinite(smax) and 0.0 < smin and smax < 0.1):
            ok = False
    lap("dequant")
    return final, ok
